# revision 1
# baseline (speedup 1.0000x reference)
"""Trainium2 Bass kernel for nn_AttentiveReadIn.

Strategy: shard the sender dim V across 8 cores (sequence parallel).
The per-receiver key/value modulation is folded algebraically into the
query / output side so the huge (b,v,u,·) tensors are never
materialized:

  scores(r,h,v) = sum_i [ (q_h @ Wk_h) * scale_k ](r,h,i) * s_ln(v,i)
  ctx(r,h,i)    = sum_v exp(scores)(r,h,v) * s_ln(v,i)
  msg(r,o)      = sum_i ctx_norm(r,h(o),i) * scale_v(r,i) * Wv(o,i) + bv

Each core computes partial [ctx | sum_exp] over its V-shard; one
AllReduce combines them; the small tail (value fold, exit proj, FFN)
runs redundantly on every core.  Scores are bounded (|s| < ~5) so
softmax is computed without max subtraction, which the AllReduce of
plain sums relies on.  Key bias bk shifts scores uniformly over v and
cancels in softmax exactly, so it is dropped.

All matmuls run in float32r (verified ~1.6e-4 worst-case rel error on
HW, final output error ~1e-4 vs the fp32 reference).
"""

import numpy as np

import concourse.bass as bass
import concourse.mybir as mybir
import concourse.tile as tile
from concourse import bacc, bass_utils
from concourse.masks import make_identity

B, U, V = 2, 32, 2048
IN, ST, CODE = 256, 512, 256
H, HD = 8, 64
INNER = H * HD
HID = ST
N_CORES = 8
R = B * U                      # 64 receiver rows
VC = V // N_CORES              # 256 senders per core per batch
S = B * VC                     # 512 sender rows per core
EPS = 1e-5

F32 = mybir.dt.float32
MMDT = mybir.dt.float32r       # matmul operand dtype
AXIS = mybir.AluOpType

# all matmul-operand weights packed into one (128, k*D) DMA, score-path first
MEGA = [("codesT", 2, 64), ("CqT", 2, 512), ("CkT", 2, 256),
        ("WqT", 4, 512), ("Wk", 4, 256), ("CvT", 2, 256), ("CeT", 2, 512),
        ("WvT", 2, 512), ("C1T", 2, 512), ("C2T", 2, 512),
        ("W1T", 4, 512), ("W2T", 4, 512)]
MEGA_F = sum(k * d for _, k, d in MEGA)


def _build(nc):
    d = {}
    def din(name, shape, dt=F32):
        d[name] = nc.dram_tensor(name, list(shape), dt, kind="ExternalInput")
        return d[name]

    din("send", (128, 4, IN))            # per-core sender shard (part-major)
    din("recv", (R, ST))
    din("mega", (128, MEGA_F), MMDT)
    din("WeT8", (64, 8, ST), MMDT)
    din("pack64", (R, 10, ST))           # host-tiled per-row vector params
    din("pack128", (128, 2, IN))
    din("bv8", (64, 8))
    out = nc.dram_tensor("out", [R, ST], F32, kind="ExternalOutput")

    from contextlib import ExitStack
    with tile.TileContext(nc) as tc, ExitStack() as es:
        wpool = es.enter_context(tc.tile_pool(name="w", bufs=1))
        apool = es.enter_context(tc.tile_pool(name="a", bufs=1))
        tpool = es.enter_context(tc.tile_pool(name="t", bufs=3))
        ps_sc = es.enter_context(tc.tile_pool(name="ps_sc", bufs=2, space="PSUM"))
        ps_ctx = es.enter_context(tc.tile_pool(name="ps_ctx", bufs=4, space="PSUM"))
        ps_g = es.enter_context(tc.tile_pool(name="ps_g", bufs=2, space="PSUM"))
        dpool = es.enter_context(tc.tile_pool(name="dram", bufs=1, space="DRAM"))

        def sb(pool, name, shape, dt=F32):
            return pool.tile(list(shape), dt, tag=name, name=name)

        # ---- load everything ----
        def load(name, shape, dt=F32, perm=None):
            sb_shape = [shape[p] for p in perm] if perm else list(shape)
            t = sb(wpool, name, sb_shape, dt)
            ap = d[name].ap()
            if perm is not None:
                ap = ap.transpose(perm)
            nc.sync.dma_start(out=t[:], in_=ap)
            return t

        send = load("send", (128, 4, IN))
        recv = load("recv", (R, ST))
        mega = sb(wpool, "mega", (128, MEGA_F), MMDT)
        _sc_end = sum(k * d for nm, k, d in MEGA
                      if nm in ("codesT", "CqT", "CkT", "WqT", "Wk"))
        _val_end = _sc_end + sum(k * d for nm, k, d in MEGA
                                 if nm in ("CvT", "CeT", "WvT"))
        nc.sync.dma_start(out=mega[:, :_sc_end],
                          in_=d["mega"].ap()[:, :_sc_end])
        pack128 = load("pack128", (128, 2, IN))
        pack64 = load("pack64", (R, 10, ST))
        nc.sync.dma_start(out=mega[:, _sc_end:_val_end],
                          in_=d["mega"].ap()[:, _sc_end:_val_end])
        WeT8 = load("WeT8", (64, 8, ST), MMDT)
        bv8 = load("bv8", (64, 8))
        nc.sync.dma_start(out=mega[:, _val_end:],
                          in_=d["mega"].ap()[:, _val_end:])
        _views, _off = {}, 0
        for _nm, _k, _d in MEGA:
            _views[_nm] = mega[:, _off:_off + _k * _d].rearrange(
                "p (k d) -> p k d", k=_k)
            _off += _k * _d
        codesT, CqT, CkT = _views["codesT"], _views["CqT"], _views["CkT"]
        WqT, Wk, CvT, CeT = _views["WqT"], _views["Wk"], _views["CvT"], _views["CeT"]
        WvT, C1T, C2T = _views["WvT"], _views["C1T"], _views["C2T"]
        W1T, W2T = _views["W1T"], _views["W2T"]
        _p64 = ["ln_r_g", "ln_r_b", "ln_f_g", "ln_f_b", "bq", "be",
                "b1", "b2", "ls_attn", "ls_ffn"]
        bc = {nm: pack64[:, j, :] for j, nm in enumerate(_p64)}
        bc["ln_s_g"] = pack128[:, 0, :]
        bc["ln_s_b"] = pack128[:, 1, :]

        epst = sb(wpool, "epst", (128, 1))
        nc.vector.memset(epst[:], EPS)
        ident32 = sb(wpool, "ident32", (128, 128), F32)
        make_identity(nc, ident32[:])
        ident = sb(wpool, "ident", (128, 128), MMDT)
        nc.vector.tensor_copy(out=ident[:], in_=ident32[:])
        ones_col = sb(wpool, "ones_col", (128, 1))
        nc.vector.memset(ones_col[:], 1.0)
        zeros32 = sb(wpool, "zeros32", (128, 32))
        nc.vector.memset(zeros32[:], 0.0)

        def transpose(dst_ps, src_ap):
            p = src_ap.shape[0]
            idt = ident if src_ap.dtype == MMDT else ident32
            nc.tensor.transpose(dst_ps.bitcast(src_ap.dtype), src_ap,
                                idt[:p, :p])

        # ---- scales: natural (R, D) = 1 + codes @ C^T ----
        scales = {}
        for nm, CT, D in [("q", CqT, ST), ("k", CkT, IN), ("v", CvT, IN),
                          ("e", CeT, ST), ("f1", C1T, ST), ("f2", C2T, HID)]:
            p = sb(ps_g, "g", (R, 512))
            for j in range(2):
                nc.tensor.matmul(p[:, :D], codesT[:, j, :], CT[:, j, :],
                                 start=(j == 0), stop=(j == 1))
            s = sb(apool, "scale_" + nm, (R, D))
            nc.scalar.add(out=s[:], in_=p[:, :D], add=1.0)
            scales[nm] = s

        # ---- receiver layernorm + x_q ----
        mvr = sb(apool, "mvr", (R, 2))
        bnr = sb(apool, "bnr", (R, 6))
        nc.vector.bn_stats(out=bnr[:], in_=recv[:])
        nc.vector.bn_aggr(out=mvr[:], in_=bnr[:])
        rstd_r = sb(apool, "rstd_r", (R, 1))
        nc.scalar.activation(out=rstd_r[:], in_=mvr[:, 1:2],
                             func=mybir.ActivationFunctionType.Sqrt, bias=epst[:R])
        nc.vector.reciprocal(out=rstd_r[:], in_=rstd_r[:])
        zr = sb(apool, "zr", (R, ST))
        nc.vector.tensor_scalar(out=zr[:], in0=recv[:], scalar1=mvr[:, 0:1],
                                scalar2=rstd_r[:], op0=AXIS.subtract, op1=AXIS.mult)
        nc.vector.tensor_mul(out=zr[:], in0=zr[:], in1=bc["ln_r_g"])
        nc.vector.tensor_add(out=zr[:], in0=zr[:], in1=bc["ln_r_b"])
        xq = sb(apool, "xq", (R, ST), MMDT)
        nc.vector.tensor_mul(out=xq[:], in0=zr[:], in1=scales["q"][:])

        # ---- q = xq @ Wq^T  (via xq^T stationary) ----
        xqT = sb(apool, "xqT", (128, 4, R), MMDT)
        for t in range(4):
            p = sb(ps_g, "g", (128, 128))
            transpose(p[:, :R], xq[:, t * 128:(t + 1) * 128])
            nc.vector.tensor_copy(out=xqT[:, t, :], in_=p[:, :R])
        qps = sb(ps_g, "g", (R, INNER))
        for t in range(4):
            nc.tensor.matmul(qps[:], xqT[:, t, :], WqT[:, t, :],
                             start=(t == 0), stop=(t == 3))
        q_sb = sb(apool, "q_sb", (R, INNER), MMDT)
        nc.vector.tensor_add(out=q_sb[:], in0=qps[:], in1=bc["bq"])
        qT = sb(apool, "qT", (128, 4, R), MMDT)
        for t in range(4):
            p = sb(ps_g, "g", (128, 128))
            transpose(p[:, :R], q_sb[:, t * 128:(t + 1) * 128])
            nc.vector.tensor_copy(out=qT[:, t, :], in_=p[:, :R])

        # ---- scale_k^T ----
        skT = sb(apool, "skT", (128, 2, R), MMDT)
        for c in range(2):
            p = sb(ps_g, "g", (128, 128))
            transpose(p[:, :R], scales["k"][:, c * 128:(c + 1) * 128])
            nc.vector.tensor_copy(out=skT[:, c, :], in_=p[:, :R])

        # ---- qkT(i,(h,r)) = sum_d Wk((h,d),i) qT((h,d),r), * scale_kT ----
        qkT = sb(apool, "qkT", (128, 2, H, R), MMDT)
        for c in range(2):
            for h in range(H):
                t, o = h // 2, (h % 2) * 64
                p = sb(ps_g, "g", (128, R))
                nc.tensor.matmul(p[:],
                                 Wk[o:o + 64, t, c * 128:(c + 1) * 128],
                                 qT[o:o + 64, t, :], start=True, stop=True)
                nc.vector.tensor_mul(out=qkT[:, c, h, :], in0=p[:],
                                     in1=skT[:, c, :])

        # ---- sender layernorm (natural) + ones column ----
        slna = sb(apool, "slna", (128, 4, IN + 2), MMDT)
        for t in range(4):
            bns = sb(tpool, "bns", (128, 6))
            mvs = sb(tpool, "mvs", (128, 2))
            nc.vector.bn_stats(out=bns[:], in_=send[:, t, :])
            nc.vector.bn_aggr(out=mvs[:], in_=bns[:])
            rstd = sb(tpool, "rstd_s", (128, 1))
            nc.scalar.activation(out=rstd[:], in_=mvs[:, 1:2],
                                 func=mybir.ActivationFunctionType.Sqrt, bias=epst[:])
            nc.vector.reciprocal(out=rstd[:], in_=rstd[:])
            zs = sb(tpool, "zs", (128, IN))
            nc.vector.tensor_scalar(out=zs[:], in0=send[:, t, :],
                                    scalar1=mvs[:, 0:1], scalar2=rstd[:],
                                    op0=AXIS.subtract, op1=AXIS.mult)
            nc.vector.tensor_mul(out=zs[:], in0=zs[:], in1=bc["ln_s_g"])
            nc.vector.tensor_add(out=slna[:, t, :IN], in0=zs[:], in1=bc["ln_s_b"])
            nc.vector.tensor_copy(out=slna[:, t, IN:],
                                  in_=ones_col.broadcast_to([128, 2]))

        # ---- s_ln^T (i, s) ----
        slnT = sb(apool, "slnT", (128, 2, S), MMDT)
        for c in range(2):
            for t in range(4):
                p = sb(ps_g, "g", (128, 128))
                transpose(p[:], slna[:, t, c * 128:(c + 1) * 128])
                nc.vector.tensor_copy(out=slnT[:, c, t * 128:(t + 1) * 128],
                                      in_=p[:])

        # ---- scoresT(s,(h,r)) -> exp -> mask ----
        eT = sb(apool, "eT", (128, 4, H * R), MMDT)
        for t in range(4):
            p = sb(ps_sc, "ps_scores", (128, H * R))
            for c in range(2):
                nc.tensor.matmul(p[:], slnT[:, c, t * 128:(t + 1) * 128],
                                 qkT[:, c, :, :].rearrange("p h r -> p (h r)"),
                                 start=(c == 0), stop=(c == 1))
            nc.scalar.activation(out=eT[:, t, :], in_=p[:],
                                 func=mybir.ActivationFunctionType.Exp,
                                 scale=float(1.0 / np.sqrt(HD)))
            # rows of tile t are batch (t // 2); zero the other batch's r cols
            off = 32 if t < 2 else 0
            nc.vector.tensor_copy(
                out=eT[:, t, :].rearrange("p (h r) -> p h r", h=H)[:, :, off:off + 32],
                in_=zeros32.unsqueeze(1).broadcast_to([128, H, 32]))

        # ---- ctx(m) = sum_t eT_t^T @ [s_ln | 1]  -> (128,(IN+1)) x4 ----
        ar_in = dpool.tile([4, 128, IN + 2], F32, tag="ar_in", name="ar_in")
        ar_out = dpool.tile([4, 128, IN + 2], F32, tag="ar_out", name="ar_out")
        # ---- pre-AR prep that only needs scales ----
        sv_rep = sb(apool, "sv_rep", (128, IN))
        nc.vector.tensor_copy(out=sv_rep[:R, :], in_=scales["v"][:])
        nc.vector.tensor_copy(out=sv_rep[R:, :], in_=scales["v"][:])
        seT8 = sb(apool, "seT8", (64, H, R), MMDT)
        for h in range(H):
            p = sb(ps_g, "g", (128, 128))
            transpose(p[:64, :R], scales["e"][:, h * 64:(h + 1) * 64])
            nc.vector.tensor_copy(out=seT8[:, h, :], in_=p[:64, :R])

        ctx_stage = sb(apool, "ctx_stage", (128, 4, IN + 2))
        for m in range(4):
            p = sb(ps_ctx, "ps_ctx", (128, IN + 2))
            for t in range(4):
                nc.tensor.matmul(p[:], eT[:, t, m * 128:(m + 1) * 128],
                                 slna[:, t, :], start=(t == 0), stop=(t == 3))
            nc.vector.tensor_copy(out=ctx_stage[:, m, :], in_=p[:])
            nc.sync.dma_start(out=ar_in[m], in_=ctx_stage[:, m, :])

        import os as _os
        if _os.environ.get("NO_COLL") == "1":
            nc.sync.dma_start(out=ar_out[:], in_=ar_in[:])
        else:
            nc.gpsimd.collective_compute(
                "AllReduce", AXIS.add,
                replica_groups=[list(range(N_CORES))],
                ins=[ar_in.opt()], outs=[ar_out.opt()])

        # ---- post-AR: normalize, * scale_v, transpose ----
        ctx_sv = sb(apool, "ctx_sv", (128, 4, IN), MMDT)
        csall = sb(apool, "csall", (128, 4, IN + 2))
        nc.sync.dma_start(out=csall[:], in_=ar_out.transpose([1, 0, 2]))
        for m in range(4):
            rz = sb(tpool, "rz", (128, 1))
            nc.vector.reciprocal(out=rz[:], in_=csall[:, m, IN:IN + 1])
            nc.vector.scalar_tensor_tensor(out=ctx_sv[:, m, :], in0=csall[:, m, :IN],
                                           scalar=rz[:], in1=sv_rep[:],
                                           op0=AXIS.mult, op1=AXIS.mult)
        ctxT = sb(apool, "ctxT", (128, 2, H * R), MMDT)
        for c in range(2):
            for m in range(4):
                p = sb(ps_g, "g", (128, 128))
                transpose(p[:], ctx_sv[:, m, c * 128:(c + 1) * 128])
                nc.vector.tensor_copy(out=ctxT[:, c, m * 128:(m + 1) * 128],
                                      in_=p[:])

        # ---- msg8(hd, h, r) = sum_i WvT(i,(h,hd)) ctxT(i,(h,r)); +bv, *scale_e ----
        msg8 = sb(apool, "msg8", (64, H, R), MMDT)
        for h in range(H):
            p = sb(ps_g, "g", (64, R))
            for c in range(2):
                nc.tensor.matmul(p[:], WvT[:, c, h * 64:(h + 1) * 64],
                                 ctxT[:, c, h * R:(h + 1) * R],
                                 start=(c == 0), stop=(c == 1))
            nc.vector.scalar_tensor_tensor(out=msg8[:, h, :], in0=p[:],
                                           scalar=bv8[:, h:h + 1], in1=seT8[:, h, :],
                                           op0=AXIS.add, op1=AXIS.mult)

        # ---- exit proj + ls_attn ----
        yps = sb(ps_g, "g", (R, ST))
        for h in range(H):
            nc.tensor.matmul(yps[:], msg8[:, h, :], WeT8[:, h, :],
                             start=(h == 0), stop=(h == H - 1))
        x_att = sb(apool, "x_att", (R, ST))
        nc.vector.tensor_add(out=x_att[:], in0=yps[:], in1=bc["be"])
        nc.vector.tensor_mul(out=x_att[:], in0=x_att[:], in1=bc["ls_attn"])

        # ---- FFN ----
        bnf = sb(apool, "bnf", (R, 6))
        mvf = sb(apool, "mvf", (R, 2))
        nc.vector.bn_stats(out=bnf[:], in_=x_att[:])
        nc.vector.bn_aggr(out=mvf[:], in_=bnf[:])
        rstd_f = sb(apool, "rstd_f", (R, 1))
        nc.scalar.activation(out=rstd_f[:], in_=mvf[:, 1:2],
                             func=mybir.ActivationFunctionType.Sqrt, bias=epst[:R])
        nc.vector.reciprocal(out=rstd_f[:], in_=rstd_f[:])
        zf = sb(apool, "zf", (R, ST))
        nc.vector.tensor_scalar(out=zf[:], in0=x_att[:], scalar1=mvf[:, 0:1],
                                scalar2=rstd_f[:], op0=AXIS.subtract, op1=AXIS.mult)
        nc.vector.tensor_mul(out=zf[:], in0=zf[:], in1=bc["ln_f_g"])
        nc.vector.tensor_add(out=zf[:], in0=zf[:], in1=bc["ln_f_b"])
        x1 = sb(apool, "x1", (R, ST), MMDT)
        nc.vector.tensor_mul(out=x1[:], in0=zf[:], in1=scales["f1"][:])
        x1T = sb(apool, "x1T", (128, 4, R), MMDT)
        for t in range(4):
            p = sb(ps_g, "g", (128, 128))
            transpose(p[:, :R], x1[:, t * 128:(t + 1) * 128])
            nc.vector.tensor_copy(out=x1T[:, t, :], in_=p[:, :R])
        h1ps = sb(ps_g, "g", (R, HID))
        for t in range(4):
            nc.tensor.matmul(h1ps[:], x1T[:, t, :], W1T[:, t, :],
                             start=(t == 0), stop=(t == 3))
        h1b = sb(apool, "h1b", (R, HID))
        nc.vector.tensor_add(out=h1b[:], in0=h1ps[:], in1=bc["b1"])
        h1g = sb(apool, "h1g", (R, HID))
        import os as _os2
        _gelu = (mybir.ActivationFunctionType.Identity
                 if _os2.environ.get("SIM_GELU_ID") == "1"
                 else mybir.ActivationFunctionType.Gelu)
        nc.scalar.activation(out=h1g[:], in_=h1b[:], func=_gelu)
        h1s = sb(apool, "h1s", (R, HID), MMDT)
        nc.vector.tensor_mul(out=h1s[:], in0=h1g[:], in1=scales["f2"][:])
        h1sT = sb(apool, "h1sT", (128, 4, R), MMDT)
        for t in range(4):
            p = sb(ps_g, "g", (128, 128))
            transpose(p[:, :R], h1s[:, t * 128:(t + 1) * 128])
            nc.vector.tensor_copy(out=h1sT[:, t, :], in_=p[:, :R])
        h2ps = sb(ps_g, "g", (R, ST))
        for t in range(4):
            nc.tensor.matmul(h2ps[:], h1sT[:, t, :], W2T[:, t, :],
                             start=(t == 0), stop=(t == 3))
        o_sb = sb(apool, "o_sb", (R, ST))
        nc.vector.tensor_add(out=o_sb[:], in0=h2ps[:], in1=bc["b2"])
        nc.vector.tensor_mul(out=o_sb[:], in0=o_sb[:], in1=bc["ls_ffn"])
        nc.vector.tensor_add(out=o_sb[:], in0=o_sb[:], in1=x_att[:])
        nc.sync.dma_start(out=out.ap(), in_=o_sb[:])

    nc.compile()
    return nc


_NC_CACHE = None


def _get_nc():
    global _NC_CACHE
    if _NC_CACHE is None:
        nc = bacc.Bacc("TRN2", target_bir_lowering=False, debug=False,
                       num_devices=N_CORES)
        _NC_CACHE = _build(nc)
    return _NC_CACHE


def make_in_maps(inputs):
    f = lambda x: np.ascontiguousarray(np.asarray(x), dtype=np.float32)
    i = {k: np.asarray(v) for k, v in inputs.items()}
    pm = lambda x: f(np.transpose(x, (1, 0, 2)))      # (k,128,D)->(128,k,D)
    pack64 = np.stack([np.asarray(i[nm], np.float32) for nm in
                       ["ln_r_g", "ln_r_b", "ln_f_g", "ln_f_b", "bq", "be",
                        "b1", "b2", "ls_attn", "ls_ffn"]])          # (10, 512)
    pack128 = np.stack([np.asarray(i["ln_s_g"], np.float32),
                        np.asarray(i["ln_s_b"], np.float32)])       # (2, 256)
    parts = {
        "codesT": pm(i["receiver_codes"].reshape(R, CODE).T.reshape(2, 128, R)),
        "CqT": pm(i["Cq"].T.reshape(2, 128, ST)),
        "CkT": pm(i["Ck"].T.reshape(2, 128, IN)),
        "CvT": pm(i["Cv"].T.reshape(2, 128, IN)),
        "CeT": pm(i["Ce"].T.reshape(2, 128, ST)),
        "C1T": pm(i["C1"].T.reshape(2, 128, ST)),
        "C2T": pm(i["C2"].T.reshape(2, 128, HID)),
        "WqT": pm(i["Wq"].T.reshape(4, 128, INNER)),
        "Wk": pm(i["Wk"].reshape(4, 128, IN)),
        "WvT": pm(i["Wv"].T.reshape(2, 128, INNER)),
        "W1T": pm(i["W1"].T.reshape(4, 128, HID)),
        "W2T": pm(i["W2"].T.reshape(4, 128, ST)),
    }
    mega = np.concatenate([parts[nm].reshape(128, -1) for nm, _, _ in MEGA],
                          axis=1)
    assert mega.shape == (128, MEGA_F)
    common = {
        "recv": f(i["receiver_states"].reshape(R, ST)),
        "mega": f(mega),
        "WeT8": pm(i["We"].T.reshape(8, 64, ST)),
        "bv8": f(i["bv"].reshape(8, 64).T),
        "pack64": f(np.broadcast_to(pack64[None], (R, 10, ST))),
        "pack128": f(np.broadcast_to(pack128[None], (128, 2, IN))),
    }
    in_maps = []
    for c in range(N_CORES):
        m = dict(common)
        shard = i["sender_states"][:, c * VC:(c + 1) * VC, :]     # (B, VC, IN)
        m["send"] = pm(shard.reshape(S, IN).reshape(4, 128, IN))
        in_maps.append(m)
    return in_maps


def kernel(**inputs) -> np.ndarray:
    nc = _get_nc()
    in_maps = make_in_maps(inputs)
    res = bass_utils.run_bass_kernel_spmd(nc, in_maps,
                                          core_ids=list(range(N_CORES)))
    return res.results[0]["out"].reshape(B, U, ST).astype(np.float32)



# revision 14
# speedup vs baseline: 1.1640x; 1.1640x over previous
"""Trainium2 Bass kernel for nn_AttentiveReadIn (v2).

Strategy: shard the sender dim V across 8 cores (sequence parallel).
The per-receiver key/value modulation is folded algebraically into the
query / output side so the huge (b,v,u,.) tensors are never
materialized:

  scores(r,h,v) = sum_i [ (q_h @ Wk_h) * scale_k ](r,h,i) * s_ln(v,i)
  ctx(r,h,i)    = sum_v exp(scores)(r,h,v) * s_ln(v,i)
  msg(r,(h,d))  = sum_i ctx(r,h,i) * scale_v(r,i) * Wv((h,d),i)

v2 changes vs v1:
  - all matmul operands in fp16 (validated 6.7e-4 rel err on host sim);
    exp is computed with a -4*ln2 bias (cancels in softmax) so the
    summed exponentials stay in fp16 range.
  - batch-compact score layout: senders only score against their own
    batch's receivers (halves the eT/ctx matmul columns, no masking).
  - the scale_v fold + Wv projection run BEFORE the AllReduce, so the
    collective carries (65, 512) f32 = 133KB (msg partial + sumexp row)
    instead of 528KB of raw ctx.
  - scale_k / scale_v / scale_e are computed directly in transposed
    layout from C^T slices (no tensor-engine transposes for them).
  - ls_attn is folded into We/be on the host; biases enter via K=1
    ones-row matmuls instead of vector adds.

Debug knobs (env): NO_COLL=1 replaces the AllReduce with a local copy;
SIM_GELU_ID=1 swaps gelu for identity; KTEST=1 drops the ones-row
matmuls; KCUT=n truncates the kernel after stage n (bisection).
"""

import os as _osK

import numpy as np

import concourse.bass as bass
import concourse.mybir as mybir
import concourse.tile as tile
from concourse import bacc, bass_utils
from concourse.masks import make_identity

B, U, V = 2, 32, 2048
IN, ST, CODE = 256, 512, 256
H, HD = 8, 64
INNER = H * HD
HID = ST
N_CORES = 8
R = B * U                      # 64 receiver rows
VC = V // N_CORES              # 256 senders per core per batch
S = B * VC                     # 512 sender rows per core
EPS = 1e-5
SHIFT = float(-4.0 * np.log(2.0))   # exp bias; cancels in softmax

F32 = mybir.dt.float32
MMDT = mybir.dt.float16        # matmul operand dtype
NPDT = np.float16
AXIS = mybir.AluOpType

# all matmul-operand weights packed into one (128, k*D) DMA, score-path first
MEGA = [("codesT", 2, 64), ("CqT", 2, 512), ("CkT", 2, 256),
        ("WqT", 4, 512), ("Wk", 4, 256), ("CvT", 2, 256), ("CeT", 2, 512),
        ("WvT", 2, 512), ("C1T", 2, 512), ("C2T", 2, 512),
        ("W1T", 4, 512), ("W2T", 4, 512)]
MEGA_F = sum(k * d for _, k, d in MEGA)


class _Cut(Exception):
    pass


def _build(nc):
    KT1 = _osK.environ.get("KTEST", "0") == "1"
    KCUT = int(_osK.environ.get("KCUT", "0"))
    d = {}
    def din(name, shape, dt=MMDT):
        d[name] = nc.dram_tensor(name, list(shape), dt, kind="ExternalInput")
        return d[name]

    din("send", (128, 4, IN))            # per-core sender shard (part-major)
    din("recv", (R, ST))
    din("mega", (128, MEGA_F))
    din("WeT8", (64, 8, ST))             # ls_attn folded into ST cols
    din("pack64", (R, 5, ST))            # ln_r_g/b, ln_f_g/b, ls_ffn
    din("pack128", (128, 2, IN))         # ln_s_g/b
    din("brow", (1, 4, ST))              # bq, be*ls_attn, b1, b2
    din("bvexp", (64, 8, 64))            # bv as (hd, h, r)
    out = nc.dram_tensor("out", [R, ST], F32, kind="ExternalOutput")

    from contextlib import ExitStack
    with tile.TileContext(nc) as tc, ExitStack() as es:
        wpool = es.enter_context(tc.tile_pool(name="w", bufs=1))
        apool = es.enter_context(tc.tile_pool(name="a", bufs=1))
        tpool = es.enter_context(tc.tile_pool(name="t", bufs=3))
        ps_g = es.enter_context(tc.tile_pool(name="ps_g", bufs=2, space="PSUM"))
        ps_sc = es.enter_context(tc.tile_pool(name="ps_sc", bufs=2, space="PSUM"))
        ps_z = es.enter_context(tc.tile_pool(name="ps_z", bufs=1, space="PSUM"))
        dpool = es.enter_context(tc.tile_pool(name="dram", bufs=1, space="DRAM"))

        def sb(pool, name, shape, dt=F32, bufs=None):
            return pool.tile(list(shape), dt, tag=name, name=name, bufs=bufs)

        def cut(k):
            if KCUT == k:
                dbg = sb(apool, "dbg", (R, ST))
                nc.vector.memset(dbg[:], 0.0)
                nc.sync.dma_start(out=out.ap(), in_=dbg[:])
                raise _Cut()

        try:
            _kbody(nc, d, out, KT1, sb, cut, wpool, apool, tpool,
                   ps_g, ps_sc, ps_z, dpool)
        except _Cut:
            pass

    nc.compile()
    return nc


def _kbody(nc, d, out, KT1, sb, cut, wpool, apool, tpool,
           ps_g, ps_sc, ps_z, dpool):
    # ---- load everything ----
    def load(name, shape, dt=MMDT):
        t = sb(wpool, name, list(shape), dt)
        nc.sync.dma_start(out=t[:], in_=d[name].ap())
        return t

    send = load("send", (128, 4, IN))
    recv = load("recv", (R, ST))
    mega = sb(wpool, "mega", (128, MEGA_F), MMDT)
    _sc_end = sum(k * dd for nm, k, dd in MEGA
                  if nm in ("codesT", "CqT", "CkT", "WqT", "Wk"))
    _val_end = _sc_end + sum(k * dd for nm, k, dd in MEGA
                             if nm in ("CvT", "CeT", "WvT"))
    nc.sync.dma_start(out=mega[:, :_sc_end],
                      in_=d["mega"].ap()[:, :_sc_end])
    pack128 = load("pack128", (128, 2, IN))
    pack64 = load("pack64", (R, 5, ST))
    brow = load("brow", (1, 4, ST))
    nc.sync.dma_start(out=mega[:, _sc_end:_val_end],
                      in_=d["mega"].ap()[:, _sc_end:_val_end])
    WeT8 = load("WeT8", (64, 8, ST))
    bvexp = load("bvexp", (64, 8, 64))
    nc.sync.dma_start(out=mega[:, _val_end:],
                      in_=d["mega"].ap()[:, _val_end:])
    _views, _off = {}, 0
    for _nm, _k, _d in MEGA:
        _views[_nm] = mega[:, _off:_off + _k * _d].rearrange(
            "p (k d) -> p k d", k=_k)
        _off += _k * _d
    codesT, CqT, CkT = _views["codesT"], _views["CqT"], _views["CkT"]
    WqT, Wk, CvT, CeT = _views["WqT"], _views["Wk"], _views["CvT"], _views["CeT"]
    WvT, C1T, C2T = _views["WvT"], _views["C1T"], _views["C2T"]
    W1T, W2T = _views["W1T"], _views["W2T"]
    _p64 = ["ln_r_g", "ln_r_b", "ln_f_g", "ln_f_b", "ls_ffn"]
    bc = {nm: pack64[:, j, :] for j, nm in enumerate(_p64)}
    bc["ln_s_g"] = pack128[:, 0, :]
    bc["ln_s_b"] = pack128[:, 1, :]

    epst = sb(wpool, "epst", (128, 1))
    nc.vector.memset(epst[:], EPS)
    ident32 = sb(wpool, "ident32", (128, 128), F32)
    make_identity(nc, ident32[:])
    ident = sb(wpool, "ident", (128, 128), MMDT)
    nc.vector.tensor_copy(out=ident[:], in_=ident32[:])
    onesA = sb(wpool, "onesA", (1, 64), MMDT)
    nc.vector.memset(onesA[:], 1.0)
    ones128 = sb(wpool, "ones128", (128, 1), MMDT)
    nc.vector.memset(ones128[:], 1.0)
    shiftt = sb(wpool, "shiftt", (128, 1))
    nc.vector.memset(shiftt[:], SHIFT)

    def transpose(dst_ps, src_ap):
        p = src_ap.shape[0]
        idt = ident if src_ap.dtype == MMDT else ident32
        nc.tensor.transpose(dst_ps, src_ap, idt[:p, :p])

    # ---- natural scales (R, D) = 1 + codes @ C^T : q, f1, f2 ----
    scales = {}
    for nm, CT, D in [("q", CqT, ST), ("f1", C1T, ST), ("f2", C2T, HID)]:
        p = sb(ps_g, "g", (R, 512))
        for j in range(2):
            nc.tensor.matmul(p[:, :D], codesT[:, j, :], CT[:, j, :],
                             start=(j == 0), stop=(j == 1))
        s = sb(apool, "scale_" + nm, (R, D), MMDT)
        nc.scalar.add(out=s[:], in_=p[:, :D], add=1.0)
        scales[nm] = s

    # ---- transposed scales: skT/svT (i, 2c, r), seT8 (hd, h, r) ----
    skT = sb(apool, "skT", (128, 2, R), MMDT)
    svT = sb(apool, "svT", (128, 2, R), MMDT)
    for CT, dst in [(CkT, skT), (CvT, svT)]:
        for c in range(2):
            p = sb(ps_g, "g", (128, R))
            for j in range(2):
                nc.tensor.matmul(p[:], CT[:, j, c * 128:(c + 1) * 128],
                                 codesT[:, j, :], start=(j == 0), stop=(j == 1))
            nc.scalar.add(out=dst[:, c, :], in_=p[:], add=1.0)
    seT8 = sb(apool, "seT8", (64, H, R), MMDT)
    for ic in range(4):
        p = sb(ps_g, "g", (128, R))
        for j in range(2):
            nc.tensor.matmul(p[:], CeT[:, j, ic * 128:(ic + 1) * 128],
                             codesT[:, j, :], start=(j == 0), stop=(j == 1))
        nc.scalar.add(out=seT8[:, 2 * ic, :], in_=p[:64, :], add=1.0)
        nc.scalar.add(out=seT8[:, 2 * ic + 1, :], in_=p[64:, :], add=1.0)

    cut(1)

    # ---- receiver layernorm + x_q ----
    mvr = sb(apool, "mvr", (R, 2))
    bnr = sb(apool, "bnr", (R, 6))
    nc.vector.bn_stats(out=bnr[:], in_=recv[:])
    nc.vector.bn_aggr(out=mvr[:], in_=bnr[:])
    rstd_r = sb(apool, "rstd_r", (R, 1))
    nc.scalar.activation(out=rstd_r[:], in_=mvr[:, 1:2],
                         func=mybir.ActivationFunctionType.Sqrt, bias=epst[:R])
    nc.vector.reciprocal(out=rstd_r[:], in_=rstd_r[:])
    zr = sb(apool, "zr", (R, ST))
    nc.vector.tensor_scalar(out=zr[:], in0=recv[:], scalar1=mvr[:, 0:1],
                            scalar2=rstd_r[:], op0=AXIS.subtract, op1=AXIS.mult)
    nc.vector.tensor_mul(out=zr[:], in0=zr[:], in1=bc["ln_r_g"])
    nc.vector.tensor_add(out=zr[:], in0=zr[:], in1=bc["ln_r_b"])
    xq = sb(apool, "xq", (R, ST), MMDT)
    nc.vector.tensor_mul(out=xq[:], in0=zr[:], in1=scales["q"][:])

    cut(11)

    # ---- q = xq @ Wq^T + bq (bias via ones-row matmul) ----
    xqT = sb(apool, "xqT", (128, 4, R), MMDT)
    for t in range(4):
        p = sb(ps_g, "gt", (128, 128), MMDT, bufs=1)
        transpose(p[:, :R], xq[:, t * 128:(t + 1) * 128])
        nc.vector.tensor_copy(out=xqT[:, t, :], in_=p[:, :R])

    cut(12)

    qps = sb(ps_g, "g", (R, INNER))
    for t in range(4):
        nc.tensor.matmul(qps[:], xqT[:, t, :], WqT[:, t, :],
                         start=(t == 0), stop=(KT1 and t == 3))
    if not KT1:
        nc.tensor.matmul(qps[:], onesA[:1, :], brow[:, 0, :],
                         start=False, stop=True)
    q_sb = sb(apool, "q_sb", (R, INNER), MMDT)
    nc.vector.tensor_copy(out=q_sb[:], in_=qps[:])

    cut(13)
    qT = sb(apool, "qT", (128, 4, R), MMDT)
    for t in range(4):
        p = sb(ps_g, "gt", (128, 128), MMDT, bufs=1)
        transpose(p[:, :R], q_sb[:, t * 128:(t + 1) * 128])
        nc.vector.tensor_copy(out=qT[:, t, :], in_=p[:, :R])

    cut(14)

    # ---- qkT(i,(h,r)) = [sum_d Wk((h,d),i) qT((h,d),r)] * skT ----
    qkT = sb(apool, "qkT", (128, 2, H, R), MMDT)
    for c in range(2):
        for h in range(H):
            t, o = h // 2, (h % 2) * 64
            p = sb(ps_g, "gqk", (128, R), bufs=2)
            nc.tensor.matmul(p[:],
                             Wk[o:o + 64, t, c * 128:(c + 1) * 128],
                             qT[o:o + 64, t, :], start=True, stop=True)
            nc.vector.tensor_mul(out=qkT[:, c, h, :], in0=p[:],
                                 in1=skT[:, c, :])

    cut(15)

    cut(2)

    # ---- sender layernorm (natural) ----
    slna = sb(apool, "slna", (128, 4, IN), MMDT)
    for t in range(4):
        bns = sb(tpool, "bns", (128, 6))
        mvs = sb(tpool, "mvs", (128, 2))
        nc.vector.bn_stats(out=bns[:], in_=send[:, t, :])
        nc.vector.bn_aggr(out=mvs[:], in_=bns[:])
        rstd = sb(tpool, "rstd_s", (128, 1))
        nc.scalar.activation(out=rstd[:], in_=mvs[:, 1:2],
                             func=mybir.ActivationFunctionType.Sqrt, bias=epst[:])
        nc.vector.reciprocal(out=rstd[:], in_=rstd[:])
        zs = sb(tpool, "zs", (128, IN))
        nc.vector.tensor_scalar(out=zs[:], in0=send[:, t, :],
                                scalar1=mvs[:, 0:1], scalar2=rstd[:],
                                op0=AXIS.subtract, op1=AXIS.mult)
        nc.vector.tensor_mul(out=zs[:], in0=zs[:], in1=bc["ln_s_g"])
        nc.vector.tensor_add(out=slna[:, t, :], in0=zs[:], in1=bc["ln_s_b"])

    # ---- s_ln^T (i, s) ----
    slnT = sb(apool, "slnT", (128, 2, S), MMDT)
    for c in range(2):
        for t in range(4):
            p = sb(ps_g, "gt", (128, 128), MMDT, bufs=1)
            transpose(p[:], slna[:, t, c * 128:(c + 1) * 128])
            nc.vector.tensor_copy(out=slnT[:, c, t * 128:(t + 1) * 128],
                                  in_=p[:])

    # ---- scoresT -> exp (batch-compact: tile t scores batch t//2) ----
    eT = sb(apool, "eT", (128, 4, H * U), MMDT)
    for t in range(4):
        b = t // 2
        p = sb(ps_sc, "ps_scores", (128, H * U))
        for c in range(2):
            nc.tensor.matmul(
                p[:], slnT[:, c, t * 128:(t + 1) * 128],
                qkT[:, c, :, b * U:(b + 1) * U],
                start=(c == 0), stop=(c == 1))
        nc.scalar.activation(out=eT[:, t, :], in_=p[:],
                             func=mybir.ActivationFunctionType.Exp,
                             scale=float(1.0 / np.sqrt(HD)), bias=shiftt[:])

    cut(3)

    # ---- AR buffer: rows 0-63 msg partial (hd,(h,b,u)), row 64 sumexp ----
    armsg = sb(apool, "armsg", (65, H, B, U))
    ar_in = dpool.tile([65, 512], F32, tag="ar_in", name="ar_in")
    ar_out = dpool.tile([65, 512], F32, tag="ar_out", name="ar_out")

    # Z row: zps(1, (b,h,u)) = colsum of eT
    if not KT1:
        for b in range(2):
            zps = sb(ps_z, "ps_z", (1, 256))
            for k, t in enumerate((2 * b, 2 * b + 1)):
                nc.tensor.matmul(zps[:], ones128[:],
                                 eT[:, t, :], start=(k == 0), stop=(k == 1))
            nc.vector.tensor_copy(
                out=armsg[64:65, :, b, :],
                in_=zps[:].rearrange("p (h u) -> p h u", h=8))
    else:
        nc.vector.memset(armsg[64:65, :, :, :], 1.0)

    # ---- ctxT(i, (b,h,u)) directly: slna^T stationary vs eT moving ----
    ctxTs = sb(apool, "ctxTs", (128, 2, B, H, U), MMDT)
    for c in range(2):
        for b in range(2):
            p = sb(ps_sc, "ps_scores", (128, H * U))
            for k, t in enumerate((2 * b, 2 * b + 1)):
                nc.tensor.matmul(p[:], slna[:, t, c * 128:(c + 1) * 128],
                                 eT[:, t, :], start=(k == 0), stop=(k == 1))
            nc.vector.tensor_mul(
                out=ctxTs[:, c, b, :, :],
                in0=p[:].rearrange("p (h u) -> p h u", h=H),
                in1=svT[:, c, b * U:(b + 1) * U].unsqueeze(1)
                    .broadcast_to([128, H, U]))

    # ---- msg partial: per head, Wv^T contraction ----
    for h in range(H):
        p = sb(ps_g, "g", (64, R))
        for c in range(2):
            nc.tensor.matmul(
                p[:], WvT[:, c, h * 64:(h + 1) * 64],
                ctxTs[:, c, :, h, :],
                start=(c == 0), stop=(c == 1))
        nc.vector.tensor_copy(out=armsg[:64, h, :, :]
                              .rearrange("p b u -> p (b u)"), in_=p[:])

    cut(4)

    nc.sync.dma_start(out=ar_in[:],
                      in_=armsg[:].rearrange("p h b u -> p (h b u)"))
    if _osK.environ.get("NO_COLL") == "1":
        nc.sync.dma_start(out=ar_out[:], in_=ar_in[:])
    else:
        nc.gpsimd.collective_compute(
            "AllReduce", AXIS.add,
            replica_groups=[list(range(N_CORES))],
            ins=[ar_in.opt()], outs=[ar_out.opt()])

    # ---- post-AR: normalize, +bv, *scale_e, exit proj ----
    csall = sb(apool, "csall", (65, 512))
    nc.sync.dma_start(out=csall[:], in_=ar_out[:])
    zrec = sb(apool, "zrec", (1, 512))
    nc.vector.reciprocal(out=zrec[:], in_=csall[64:65, :])
    zrec16 = sb(apool, "zrec16", (1, 512), MMDT)
    nc.vector.tensor_copy(out=zrec16[:], in_=zrec[:])
    msgn = sb(apool, "msgn", (64, 512))
    if not KT1:
        zbps = sb(ps_g, "g", (64, 512))
        nc.tensor.matmul(zbps[:], onesA[:1, :], zrec16[:],
                         start=True, stop=True)
        nc.vector.tensor_mul(out=msgn[:], in0=csall[:64, :], in1=zbps[:])
    else:
        nc.vector.tensor_copy(out=msgn[:], in_=csall[:64, :])
    nc.vector.tensor_add(out=msgn[:], in0=msgn[:],
                         in1=bvexp[:].rearrange("p h u -> p (h u)"))
    y8 = sb(apool, "y8", (64, H, R), MMDT)
    nc.vector.tensor_mul(out=y8[:].rearrange("p h u -> p (h u)"),
                         in0=msgn[:],
                         in1=seT8[:].rearrange("p h u -> p (h u)"))
    xps = sb(ps_g, "g", (R, ST))
    for h in range(H):
        nc.tensor.matmul(xps[:], y8[:, h, :], WeT8[:, h, :],
                         start=(h == 0), stop=(KT1 and h == H - 1))
    if not KT1:
        nc.tensor.matmul(xps[:], onesA[:1, :], brow[:, 1, :],
                         start=False, stop=True)
    x_att = sb(apool, "x_att", (R, ST))
    nc.vector.tensor_copy(out=x_att[:], in_=xps[:])

    cut(5)

    # ---- FFN ----
    bnf = sb(apool, "bnf", (R, 6))
    mvf = sb(apool, "mvf", (R, 2))
    nc.vector.bn_stats(out=bnf[:], in_=x_att[:])
    nc.vector.bn_aggr(out=mvf[:], in_=bnf[:])
    rstd_f = sb(apool, "rstd_f", (R, 1))
    nc.scalar.activation(out=rstd_f[:], in_=mvf[:, 1:2],
                         func=mybir.ActivationFunctionType.Sqrt, bias=epst[:R])
    nc.vector.reciprocal(out=rstd_f[:], in_=rstd_f[:])
    zf = sb(apool, "zf", (R, ST))
    nc.vector.tensor_scalar(out=zf[:], in0=x_att[:], scalar1=mvf[:, 0:1],
                            scalar2=rstd_f[:], op0=AXIS.subtract, op1=AXIS.mult)
    nc.vector.tensor_mul(out=zf[:], in0=zf[:], in1=bc["ln_f_g"])
    nc.vector.tensor_add(out=zf[:], in0=zf[:], in1=bc["ln_f_b"])
    x1 = sb(apool, "x1", (R, ST), MMDT)
    nc.vector.tensor_mul(out=x1[:], in0=zf[:], in1=scales["f1"][:])
    x1T = sb(apool, "x1T", (128, 4, R), MMDT)
    for t in range(4):
        p = sb(ps_g, "gt", (128, 128), MMDT, bufs=1)
        transpose(p[:, :R], x1[:, t * 128:(t + 1) * 128])
        nc.vector.tensor_copy(out=x1T[:, t, :], in_=p[:, :R])
    h1ps = sb(ps_g, "g", (R, HID))
    for t in range(4):
        nc.tensor.matmul(h1ps[:], x1T[:, t, :], W1T[:, t, :],
                         start=(t == 0), stop=(KT1 and t == 3))
    if not KT1:
        nc.tensor.matmul(h1ps[:], onesA[:1, :], brow[:, 2, :],
                         start=False, stop=True)
    h1g = sb(apool, "h1g", (R, HID))
    _gelu = (mybir.ActivationFunctionType.Identity
             if _osK.environ.get("SIM_GELU_ID") == "1"
             else mybir.ActivationFunctionType.Gelu)
    nc.scalar.activation(out=h1g[:], in_=h1ps[:], func=_gelu)
    h1s = sb(apool, "h1s", (R, HID), MMDT)
    nc.vector.tensor_mul(out=h1s[:], in0=h1g[:], in1=scales["f2"][:])
    h1sT = sb(apool, "h1sT", (128, 4, R), MMDT)
    for t in range(4):
        p = sb(ps_g, "gt", (128, 128), MMDT, bufs=1)
        transpose(p[:, :R], h1s[:, t * 128:(t + 1) * 128])
        nc.vector.tensor_copy(out=h1sT[:, t, :], in_=p[:, :R])
    h2ps = sb(ps_g, "g", (R, ST))
    for t in range(4):
        nc.tensor.matmul(h2ps[:], h1sT[:, t, :], W2T[:, t, :],
                         start=(t == 0), stop=(KT1 and t == 3))
    if not KT1:
        nc.tensor.matmul(h2ps[:], onesA[:1, :], brow[:, 3, :],
                         start=False, stop=True)
    o_sb = sb(apool, "o_sb", (R, ST))
    nc.vector.tensor_mul(out=o_sb[:], in0=h2ps[:], in1=bc["ls_ffn"])
    nc.vector.tensor_add(out=o_sb[:], in0=o_sb[:], in1=x_att[:])
    nc.sync.dma_start(out=out.ap(), in_=o_sb[:])


_NC_CACHE = None


def _get_nc():
    global _NC_CACHE
    if _NC_CACHE is None:
        nc = bacc.Bacc("TRN2", target_bir_lowering=False, debug=False,
                       num_devices=N_CORES)
        _NC_CACHE = _build(nc)
    return _NC_CACHE


def make_in_maps(inputs):
    f = lambda x: np.ascontiguousarray(np.asarray(x, np.float32), dtype=NPDT)
    i = {k: np.asarray(v, np.float32) for k, v in inputs.items()}
    pm = lambda x: f(np.transpose(x, (1, 0, 2)))      # (k,128,D)->(128,k,D)
    ls_a = i["ls_attn"]
    WeP = i["We"] * ls_a[:, None]                      # fold ls_attn
    pack64 = np.stack([i["ln_r_g"], i["ln_r_b"], i["ln_f_g"], i["ln_f_b"],
                       i["ls_ffn"]])                   # (5, 512)
    pack128 = np.stack([i["ln_s_g"], i["ln_s_b"]])     # (2, 256)
    brow = np.stack([i["bq"], i["be"] * ls_a, i["b1"], i["b2"]])  # (4, 512)
    parts = {
        "codesT": pm(i["receiver_codes"].reshape(R, CODE).T.reshape(2, 128, R)),
        "CqT": pm(i["Cq"].T.reshape(2, 128, ST)),
        "CkT": pm(i["Ck"].T.reshape(2, 128, IN)),
        "CvT": pm(i["Cv"].T.reshape(2, 128, IN)),
        "CeT": pm(i["Ce"].T.reshape(2, 128, ST)),
        "C1T": pm(i["C1"].T.reshape(2, 128, ST)),
        "C2T": pm(i["C2"].T.reshape(2, 128, HID)),
        "WqT": pm(i["Wq"].T.reshape(4, 128, INNER)),
        "Wk": pm(i["Wk"].reshape(4, 128, IN)),
        "WvT": pm(i["Wv"].T.reshape(2, 128, INNER)),
        "W1T": pm(i["W1"].T.reshape(4, 128, HID)),
        "W2T": pm(i["W2"].T.reshape(4, 128, ST)),
    }
    mega = np.concatenate([parts[nm].reshape(128, -1) for nm, _, _ in MEGA],
                          axis=1)
    assert mega.shape == (128, MEGA_F)
    common = {
        "recv": f(i["receiver_states"].reshape(R, ST)),
        "mega": f(mega),
        "WeT8": pm(WeP.T.reshape(8, 64, ST)),
        "pack64": f(np.broadcast_to(pack64[None], (R, 5, ST))),
        "pack128": f(np.broadcast_to(pack128[None], (128, 2, IN))),
        "brow": f(brow[None]),
        "bvexp": f(np.broadcast_to(i["bv"].reshape(8, 64).T[:, :, None],
                                   (64, 8, 64))),
    }
    in_maps = []
    for c in range(N_CORES):
        m = dict(common)
        shard = i["sender_states"][:, c * VC:(c + 1) * VC, :]     # (B, VC, IN)
        m["send"] = pm(shard.reshape(S, IN).reshape(4, 128, IN))
        in_maps.append(m)
    return in_maps


def kernel(**inputs) -> np.ndarray:
    nc = _get_nc()
    in_maps = make_in_maps(inputs)
    res = bass_utils.run_bass_kernel_spmd(nc, in_maps,
                                          core_ids=list(range(N_CORES)))
    return res.results[0]["out"].reshape(B, U, ST).astype(np.float32)


# revision 15
# speedup vs baseline: 1.2260x; 1.0532x over previous
"""Trainium2 Bass kernel for nn_AttentiveReadIn (v2).

Strategy: shard the sender dim V across 8 cores (sequence parallel).
The per-receiver key/value modulation is folded algebraically into the
query / output side so the huge (b,v,u,.) tensors are never
materialized:

  scores(r,h,v) = sum_i [ (q_h @ Wk_h) * scale_k ](r,h,i) * s_ln(v,i)
  ctx(r,h,i)    = sum_v exp(scores)(r,h,v) * s_ln(v,i)
  msg(r,(h,d))  = sum_i ctx(r,h,i) * scale_v(r,i) * Wv((h,d),i)

v2 changes vs v1:
  - all matmul operands in fp16 (validated 6.7e-4 rel err on host sim);
    exp is computed with a -4*ln2 bias (cancels in softmax) so the
    summed exponentials stay in fp16 range.
  - batch-compact score layout: senders only score against their own
    batch's receivers (halves the eT/ctx matmul columns, no masking).
  - the scale_v fold + Wv projection run BEFORE the AllReduce, so the
    collective carries (65, 512) f32 = 133KB (msg partial + sumexp row)
    instead of 528KB of raw ctx.
  - scale_k / scale_v / scale_e are computed directly in transposed
    layout from C^T slices (no tensor-engine transposes for them).
  - ls_attn is folded into We/be on the host; biases enter via K=1
    ones-row matmuls instead of vector adds.

Debug knobs (env): NO_COLL=1 replaces the AllReduce with a local copy;
SIM_GELU_ID=1 swaps gelu for identity; KTEST=1 drops the ones-row
matmuls; KCUT=n truncates the kernel after stage n (bisection).
"""

import os as _osK

import numpy as np

import concourse.bass as bass
import concourse.mybir as mybir
import concourse.tile as tile
from concourse import bacc, bass_utils
from concourse.masks import make_identity

B, U, V = 2, 32, 2048
IN, ST, CODE = 256, 512, 256
H, HD = 8, 64
INNER = H * HD
HID = ST
N_CORES = 8
R = B * U                      # 64 receiver rows
VC = V // N_CORES              # 256 senders per core per batch
S = B * VC                     # 512 sender rows per core
EPS = 1e-5
SHIFT = float(-4.0 * np.log(2.0))   # exp bias; cancels in softmax

F32 = mybir.dt.float32
MMDT = mybir.dt.float16        # matmul operand dtype
NPDT = np.float16
AXIS = mybir.AluOpType

# all matmul-operand weights packed into one (128, k*D) DMA, score-path first
MEGA = [("codesT", 2, 64), ("CqT", 2, 512), ("CkT", 2, 256),
        ("WqT", 4, 512), ("Wk", 4, 256), ("CvT", 2, 256), ("CeT", 2, 512),
        ("WvT", 2, 512), ("C1T", 2, 512), ("C2T", 2, 512),
        ("W1T", 4, 512), ("W2T", 4, 512)]
MEGA_F = sum(k * d for _, k, d in MEGA)


class _Cut(Exception):
    pass


def _build(nc):
    KT1 = _osK.environ.get("KTEST", "0") == "1"
    KCUT = int(_osK.environ.get("KCUT", "0"))
    d = {}
    def din(name, shape, dt=MMDT):
        d[name] = nc.dram_tensor(name, list(shape), dt, kind="ExternalInput")
        return d[name]

    din("send", (128, 4, IN))            # per-core sender shard (part-major)
    din("recv", (R, ST))
    din("mega", (128, MEGA_F))
    din("WeT8", (64, 8, ST))             # ls_attn folded into ST cols
    din("pack64", (R, 5, ST))            # ln_r_g/b, ln_f_g/b, ls_ffn
    din("pack128", (128, 2, IN))         # ln_s_g/b
    din("brow", (1, 4, ST))              # bq, be*ls_attn, b1, b2
    din("bvexp", (64, 8, 64))            # bv as (hd, h, r)
    out = nc.dram_tensor("out", [R, ST], F32, kind="ExternalOutput")

    from contextlib import ExitStack
    with tile.TileContext(nc) as tc, ExitStack() as es:
        wpool = es.enter_context(tc.tile_pool(name="w", bufs=1))
        apool = es.enter_context(tc.tile_pool(name="a", bufs=1))
        tpool = es.enter_context(tc.tile_pool(name="t", bufs=3))
        ps_g = es.enter_context(tc.tile_pool(name="ps_g", bufs=2, space="PSUM"))
        ps_sc = es.enter_context(tc.tile_pool(name="ps_sc", bufs=2, space="PSUM"))
        ps_z = es.enter_context(tc.tile_pool(name="ps_z", bufs=1, space="PSUM"))
        dpool = es.enter_context(tc.tile_pool(name="dram", bufs=1, space="DRAM"))

        def sb(pool, name, shape, dt=F32, bufs=None):
            return pool.tile(list(shape), dt, tag=name, name=name, bufs=bufs)

        def cut(k):
            if KCUT == k:
                dbg = sb(apool, "dbg", (R, ST))
                nc.vector.memset(dbg[:], 0.0)
                nc.sync.dma_start(out=out.ap(), in_=dbg[:])
                raise _Cut()

        try:
            _kbody(nc, d, out, KT1, sb, cut, wpool, apool, tpool,
                   ps_g, ps_sc, ps_z, dpool)
        except _Cut:
            pass

    nc.compile()
    return nc


def _kbody(nc, d, out, KT1, sb, cut, wpool, apool, tpool,
           ps_g, ps_sc, ps_z, dpool):
    # ---- load everything ----
    def load(name, shape, dt=MMDT):
        t = sb(wpool, name, list(shape), dt)
        nc.sync.dma_start(out=t[:], in_=d[name].ap())
        return t

    send = load("send", (128, 4, IN))
    recv = load("recv", (R, ST))
    mega = sb(wpool, "mega", (128, MEGA_F), MMDT)
    _c1 = sum(k * dd for nm, k, dd in MEGA
              if nm in ("codesT", "CqT", "CkT"))
    _sc_end = sum(k * dd for nm, k, dd in MEGA
                  if nm in ("codesT", "CqT", "CkT", "WqT", "Wk"))
    _val_end = _sc_end + sum(k * dd for nm, k, dd in MEGA
                             if nm in ("CvT", "CeT", "WvT"))
    pack128 = load("pack128", (128, 2, IN))
    pack64 = load("pack64", (R, 5, ST))
    brow = load("brow", (1, 4, ST))
    nc.sync.dma_start(out=mega[:, :_c1], in_=d["mega"].ap()[:, :_c1])
    nc.sync.dma_start(out=mega[:, _c1:_sc_end],
                      in_=d["mega"].ap()[:, _c1:_sc_end])
    nc.sync.dma_start(out=mega[:, _sc_end:_val_end],
                      in_=d["mega"].ap()[:, _sc_end:_val_end])
    WeT8 = load("WeT8", (64, 8, ST))
    bvexp = load("bvexp", (64, 8, 64))
    nc.sync.dma_start(out=mega[:, _val_end:],
                      in_=d["mega"].ap()[:, _val_end:])
    _views, _off = {}, 0
    for _nm, _k, _d in MEGA:
        _views[_nm] = mega[:, _off:_off + _k * _d].rearrange(
            "p (k d) -> p k d", k=_k)
        _off += _k * _d
    codesT, CqT, CkT = _views["codesT"], _views["CqT"], _views["CkT"]
    WqT, Wk, CvT, CeT = _views["WqT"], _views["Wk"], _views["CvT"], _views["CeT"]
    WvT, C1T, C2T = _views["WvT"], _views["C1T"], _views["C2T"]
    W1T, W2T = _views["W1T"], _views["W2T"]
    _p64 = ["ln_r_g", "ln_r_b", "ln_f_g", "ln_f_b", "ls_ffn"]
    bc = {nm: pack64[:, j, :] for j, nm in enumerate(_p64)}
    bc["ln_s_g"] = pack128[:, 0, :]
    bc["ln_s_b"] = pack128[:, 1, :]

    epst = sb(wpool, "epst", (128, 1))
    nc.vector.memset(epst[:], EPS)
    ident32 = sb(wpool, "ident32", (128, 128), F32)
    make_identity(nc, ident32[:])
    ident = sb(wpool, "ident", (128, 128), MMDT)
    nc.vector.tensor_copy(out=ident[:], in_=ident32[:])
    onesA = sb(wpool, "onesA", (1, 64), MMDT)
    nc.vector.memset(onesA[:], 1.0)
    ones128 = sb(wpool, "ones128", (128, 1), MMDT)
    nc.vector.memset(ones128[:], 1.0)
    shiftt = sb(wpool, "shiftt", (128, 1))
    nc.vector.memset(shiftt[:], SHIFT)

    def transpose(dst_ps, src_ap):
        p = src_ap.shape[0]
        idt = ident if src_ap.dtype == MMDT else ident32
        nc.tensor.transpose(dst_ps, src_ap, idt[:p, :p])

    # ---- natural scales (R, D) = 1 + codes @ C^T : q, f1, f2 ----
    scales = {}
    for nm, CT, D in [("q", CqT, ST), ("f1", C1T, ST), ("f2", C2T, HID)]:
        p = sb(ps_g, "g", (R, 512))
        for j in range(2):
            nc.tensor.matmul(p[:, :D], codesT[:, j, :], CT[:, j, :],
                             start=(j == 0), stop=(j == 1))
        s = sb(apool, "scale_" + nm, (R, D), MMDT)
        nc.scalar.add(out=s[:], in_=p[:, :D], add=1.0)
        scales[nm] = s

    # ---- transposed scales: skT/svT (i, 2c, r), seT8 (hd, h, r) ----
    skT = sb(apool, "skT", (128, 2, R), MMDT)
    svT = sb(apool, "svT", (128, 2, R), MMDT)
    for CT, dst in [(CkT, skT), (CvT, svT)]:
        for c in range(2):
            p = sb(ps_g, "g", (128, R))
            for j in range(2):
                nc.tensor.matmul(p[:], CT[:, j, c * 128:(c + 1) * 128],
                                 codesT[:, j, :], start=(j == 0), stop=(j == 1))
            nc.scalar.add(out=dst[:, c, :], in_=p[:], add=1.0)
    # f-LN gain/bias folded into scale_f1 (used post-AR)
    sf1g = sb(apool, "sf1g", (R, ST), MMDT)
    nc.vector.tensor_mul(out=sf1g[:], in0=scales["f1"][:], in1=bc["ln_f_g"])
    bf1 = sb(apool, "bf1", (R, ST), MMDT)
    nc.vector.tensor_mul(out=bf1[:], in0=scales["f1"][:], in1=bc["ln_f_b"])
    seT8 = sb(apool, "seT8", (64, H, R), MMDT)
    for ic in range(4):
        p = sb(ps_g, "g", (128, R))
        for j in range(2):
            nc.tensor.matmul(p[:], CeT[:, j, ic * 128:(ic + 1) * 128],
                             codesT[:, j, :], start=(j == 0), stop=(j == 1))
        nc.scalar.add(out=seT8[:, 2 * ic, :], in_=p[:64, :], add=1.0)
        nc.scalar.add(out=seT8[:, 2 * ic + 1, :], in_=p[64:, :], add=1.0)

    cut(1)

    # ---- receiver layernorm + x_q ----
    mvr = sb(apool, "mvr", (R, 2))
    bnr = sb(apool, "bnr", (R, 6))
    nc.vector.bn_stats(out=bnr[:], in_=recv[:])
    nc.vector.bn_aggr(out=mvr[:], in_=bnr[:])
    rstd_r = sb(apool, "rstd_r", (R, 1))
    nc.scalar.activation(out=rstd_r[:], in_=mvr[:, 1:2],
                         func=mybir.ActivationFunctionType.Sqrt, bias=epst[:R])
    nc.vector.reciprocal(out=rstd_r[:], in_=rstd_r[:])
    zr = sb(apool, "zr", (R, ST))
    nc.vector.tensor_scalar(out=zr[:], in0=recv[:], scalar1=mvr[:, 0:1],
                            scalar2=rstd_r[:], op0=AXIS.subtract, op1=AXIS.mult)
    nc.vector.tensor_mul(out=zr[:], in0=zr[:], in1=bc["ln_r_g"])
    nc.vector.tensor_add(out=zr[:], in0=zr[:], in1=bc["ln_r_b"])
    xq = sb(apool, "xq", (R, ST), MMDT)
    nc.vector.tensor_mul(out=xq[:], in0=zr[:], in1=scales["q"][:])

    cut(11)

    # ---- q = xq @ Wq^T + bq (bias via ones-row matmul) ----
    xqT = sb(apool, "xqT", (128, 4, R), MMDT)
    for t in range(4):
        p = sb(ps_g, "gt", (128, 128), MMDT, bufs=1)
        transpose(p[:, :R], xq[:, t * 128:(t + 1) * 128])
        nc.vector.tensor_copy(out=xqT[:, t, :], in_=p[:, :R])

    cut(12)

    qps = sb(ps_g, "g", (R, INNER))
    for t in range(4):
        nc.tensor.matmul(qps[:], xqT[:, t, :], WqT[:, t, :],
                         start=(t == 0), stop=(KT1 and t == 3))
    if not KT1:
        nc.tensor.matmul(qps[:], onesA[:1, :], brow[:, 0, :],
                         start=False, stop=True)
    q_sb = sb(apool, "q_sb", (R, INNER), MMDT)
    nc.vector.tensor_copy(out=q_sb[:], in_=qps[:])

    cut(13)
    qT = sb(apool, "qT", (128, 4, R), MMDT)
    for t in range(4):
        p = sb(ps_g, "gt", (128, 128), MMDT, bufs=1)
        transpose(p[:, :R], q_sb[:, t * 128:(t + 1) * 128])
        nc.vector.tensor_copy(out=qT[:, t, :], in_=p[:, :R])

    cut(14)

    # ---- qkT(i,(h,r)) = [sum_d Wk((h,d),i) qT((h,d),r)] * skT ----
    qkT = sb(apool, "qkT", (128, 2, H, R), MMDT)
    for c in range(2):
        for h in range(H):
            t, o = h // 2, (h % 2) * 64
            p = sb(ps_g, "gqk", (128, R), bufs=2)
            nc.tensor.matmul(p[:],
                             Wk[o:o + 64, t, c * 128:(c + 1) * 128],
                             qT[o:o + 64, t, :], start=True, stop=True)
            nc.vector.tensor_mul(out=qkT[:, c, h, :], in0=p[:],
                                 in1=skT[:, c, :])

    cut(15)

    cut(2)

    # ---- sender layernorm (natural) ----
    slna = sb(apool, "slna", (128, 4, IN), MMDT)
    for t in range(4):
        bns = sb(tpool, "bns", (128, 6))
        mvs = sb(tpool, "mvs", (128, 2))
        nc.vector.bn_stats(out=bns[:], in_=send[:, t, :])
        nc.vector.bn_aggr(out=mvs[:], in_=bns[:])
        rstd = sb(tpool, "rstd_s", (128, 1))
        nc.scalar.activation(out=rstd[:], in_=mvs[:, 1:2],
                             func=mybir.ActivationFunctionType.Sqrt, bias=epst[:])
        nc.vector.reciprocal(out=rstd[:], in_=rstd[:])
        zs = sb(tpool, "zs", (128, IN))
        nc.vector.tensor_scalar(out=zs[:], in0=send[:, t, :],
                                scalar1=mvs[:, 0:1], scalar2=rstd[:],
                                op0=AXIS.subtract, op1=AXIS.mult)
        nc.vector.tensor_mul(out=zs[:], in0=zs[:], in1=bc["ln_s_g"])
        nc.vector.tensor_add(out=slna[:, t, :], in0=zs[:], in1=bc["ln_s_b"])

    # ---- s_ln^T (i, s) ----
    slnT = sb(apool, "slnT", (128, 2, S), MMDT)
    for c in range(2):
        for t in range(4):
            p = sb(ps_g, "gt", (128, 128), MMDT, bufs=1)
            transpose(p[:], slna[:, t, c * 128:(c + 1) * 128])
            nc.vector.tensor_copy(out=slnT[:, c, t * 128:(t + 1) * 128],
                                  in_=p[:])

    # ---- scoresT -> exp (batch-compact: tile t scores batch t//2) ----
    eT = sb(apool, "eT", (128, 4, H * U), MMDT)
    for t in range(4):
        b = t // 2
        p = sb(ps_sc, "ps_scores", (128, H * U))
        for c in range(2):
            nc.tensor.matmul(
                p[:], slnT[:, c, t * 128:(t + 1) * 128],
                qkT[:, c, :, b * U:(b + 1) * U],
                start=(c == 0), stop=(c == 1))
        nc.scalar.activation(out=eT[:, t, :], in_=p[:],
                             func=mybir.ActivationFunctionType.Exp,
                             scale=float(1.0 / np.sqrt(HD)), bias=shiftt[:])

    # keep the sqrt table resident for the post-AR layernorm: touch Sqrt
    # after the last Exp so no table load lands on the tail critical path
    tdum = sb(apool, "tdum", (1, 1))
    nc.scalar.activation(out=tdum[:], in_=eT[:1, 3, :1],
                         func=mybir.ActivationFunctionType.Sqrt)

    cut(3)

    # ---- AR buffer: rows 0-63 msg partial (hd,(h,b,u)), row 64 sumexp ----
    armsg = sb(apool, "armsg", (65, H, B, U), MMDT)
    ar_in = dpool.tile([65, 512], MMDT, tag="ar_in", name="ar_in")
    ar_out = dpool.tile([65, 512], MMDT, tag="ar_out", name="ar_out")

    # Z row: zps(1, (b,h,u)) = colsum of eT
    if not KT1:
        for b in range(2):
            zps = sb(ps_z, "ps_z", (1, 256))
            for k, t in enumerate((2 * b, 2 * b + 1)):
                nc.tensor.matmul(zps[:], ones128[:],
                                 eT[:, t, :], start=(k == 0), stop=(k == 1))
            nc.vector.tensor_copy(
                out=armsg[64:65, :, b, :],
                in_=zps[:].rearrange("p (h u) -> p h u", h=8))
    else:
        nc.vector.memset(armsg[64:65, :, :, :], 1.0)

    # ---- ctxT(i, (b,h,u)) directly: slna^T stationary vs eT moving ----
    ctxTs = sb(apool, "ctxTs", (128, 2, B, H, U), MMDT)
    for c in range(2):
        for b in range(2):
            p = sb(ps_sc, "ps_scores", (128, H * U))
            for k, t in enumerate((2 * b, 2 * b + 1)):
                nc.tensor.matmul(p[:], slna[:, t, c * 128:(c + 1) * 128],
                                 eT[:, t, :], start=(k == 0), stop=(k == 1))
            nc.vector.tensor_mul(
                out=ctxTs[:, c, b, :, :],
                in0=p[:].rearrange("p (h u) -> p h u", h=H),
                in1=svT[:, c, b * U:(b + 1) * U].unsqueeze(1)
                    .broadcast_to([128, H, U]))

    # ---- msg partial: per head, Wv^T contraction ----
    for h in range(H):
        p = sb(ps_g, "g", (64, R))
        for c in range(2):
            nc.tensor.matmul(
                p[:], WvT[:, c, h * 64:(h + 1) * 64],
                ctxTs[:, c, :, h, :],
                start=(c == 0), stop=(c == 1))
        nc.vector.tensor_copy(out=armsg[:64, h, :, :]
                              .rearrange("p b u -> p (b u)"), in_=p[:])

    cut(4)

    nc.sync.dma_start(out=ar_in[:],
                      in_=armsg[:].rearrange("p h b u -> p (h b u)"))
    if _osK.environ.get("NO_COLL") == "1":
        nc.sync.dma_start(out=ar_out[:], in_=ar_in[:])
    else:
        nc.gpsimd.collective_compute(
            "AllReduce", AXIS.add,
            replica_groups=[list(range(N_CORES))],
            ins=[ar_in.opt()], outs=[ar_out.opt()])

    # ---- post-AR: normalize, +bv, *scale_e, exit proj ----
    csall = sb(apool, "csall", (65, 512), MMDT)
    nc.sync.dma_start(out=csall[:], in_=ar_out[:])
    zrec = sb(apool, "zrec", (1, 512))
    nc.vector.reciprocal(out=zrec[:], in_=csall[64:65, :])
    zrec16 = sb(apool, "zrec16", (1, 512), MMDT)
    nc.vector.tensor_copy(out=zrec16[:], in_=zrec[:])
    msgn = sb(apool, "msgn", (64, 512))
    if not KT1:
        zbps = sb(ps_g, "g", (64, 512))
        nc.tensor.matmul(zbps[:], onesA[:1, :], zrec16[:],
                         start=True, stop=True)
        nc.vector.tensor_mul(out=msgn[:], in0=csall[:64, :], in1=zbps[:])
    else:
        nc.vector.tensor_copy(out=msgn[:], in_=csall[:64, :])
    nc.vector.tensor_add(out=msgn[:], in0=msgn[:],
                         in1=bvexp[:].rearrange("p h u -> p (h u)"))
    y8 = sb(apool, "y8", (64, H, R), MMDT)
    nc.vector.tensor_mul(out=y8[:].rearrange("p h u -> p (h u)"),
                         in0=msgn[:],
                         in1=seT8[:].rearrange("p h u -> p (h u)"))
    xps = sb(ps_z, "ps_z", (R, ST), bufs=1)
    for h in range(H):
        nc.tensor.matmul(xps[:], y8[:, h, :], WeT8[:, h, :],
                         start=(h == 0), stop=(KT1 and h == H - 1))
    if not KT1:
        nc.tensor.matmul(xps[:], onesA[:1, :], brow[:, 1, :],
                         start=False, stop=True)
    x_att = xps

    cut(5)

    # ---- FFN ----
    bnf = sb(apool, "bnf", (R, 6))
    mvf = sb(apool, "mvf", (R, 2))
    nc.vector.bn_stats(out=bnf[:], in_=x_att[:])
    nc.vector.bn_aggr(out=mvf[:], in_=bnf[:])
    rstd_f = sb(apool, "rstd_f", (R, 1))
    nc.scalar.activation(out=rstd_f[:], in_=mvf[:, 1:2],
                         func=mybir.ActivationFunctionType.Sqrt, bias=epst[:R])
    nc.vector.reciprocal(out=rstd_f[:], in_=rstd_f[:])
    zf = sb(apool, "zf", (R, ST))
    nc.vector.tensor_scalar(out=zf[:], in0=x_att[:], scalar1=mvf[:, 0:1],
                            scalar2=rstd_f[:], op0=AXIS.subtract, op1=AXIS.mult)
    x1 = sb(apool, "x1", (R, ST), MMDT)
    nc.vector.tensor_mul(out=x1[:], in0=zf[:], in1=sf1g[:])
    nc.vector.tensor_add(out=x1[:], in0=x1[:], in1=bf1[:])
    x1T = sb(apool, "x1T", (128, 4, R), MMDT)
    for t in range(4):
        p = sb(ps_g, "gt", (128, 128), MMDT, bufs=1)
        transpose(p[:, :R], x1[:, t * 128:(t + 1) * 128])
        nc.vector.tensor_copy(out=x1T[:, t, :], in_=p[:, :R])
    h1ps = sb(ps_g, "g", (R, HID))
    for t in range(4):
        nc.tensor.matmul(h1ps[:], x1T[:, t, :], W1T[:, t, :],
                         start=(t == 0), stop=(KT1 and t == 3))
    if not KT1:
        nc.tensor.matmul(h1ps[:], onesA[:1, :], brow[:, 2, :],
                         start=False, stop=True)
    h1g = sb(apool, "h1g", (R, HID), MMDT)
    _gelu = (mybir.ActivationFunctionType.Identity
             if _osK.environ.get("SIM_GELU_ID") == "1"
             else mybir.ActivationFunctionType.Gelu)
    nc.scalar.activation(out=h1g[:], in_=h1ps[:], func=_gelu)
    h1s = sb(apool, "h1s", (R, HID), MMDT)
    nc.vector.tensor_mul(out=h1s[:], in0=h1g[:], in1=scales["f2"][:])
    h1sT = sb(apool, "h1sT", (128, 4, R), MMDT)
    for t in range(4):
        p = sb(ps_g, "gt", (128, 128), MMDT, bufs=1)
        transpose(p[:, :R], h1s[:, t * 128:(t + 1) * 128])
        nc.vector.tensor_copy(out=h1sT[:, t, :], in_=p[:, :R])
    h2ps = sb(ps_g, "g", (R, ST))
    for t in range(4):
        nc.tensor.matmul(h2ps[:], h1sT[:, t, :], W2T[:, t, :],
                         start=(t == 0), stop=(KT1 and t == 3))
    if not KT1:
        nc.tensor.matmul(h2ps[:], onesA[:1, :], brow[:, 3, :],
                         start=False, stop=True)
    o_sb = sb(apool, "o_sb", (R, ST))
    nc.vector.tensor_mul(out=o_sb[:], in0=h2ps[:], in1=bc["ls_ffn"])
    nc.vector.tensor_add(out=o_sb[:], in0=o_sb[:], in1=x_att[:])
    nc.sync.dma_start(out=out.ap(), in_=o_sb[:])


_NC_CACHE = None


def _get_nc():
    global _NC_CACHE
    if _NC_CACHE is None:
        nc = bacc.Bacc("TRN2", target_bir_lowering=False, debug=False,
                       num_devices=N_CORES)
        _NC_CACHE = _build(nc)
    return _NC_CACHE


def make_in_maps(inputs):
    f = lambda x: np.ascontiguousarray(np.asarray(x, np.float32), dtype=NPDT)
    i = {k: np.asarray(v, np.float32) for k, v in inputs.items()}
    pm = lambda x: f(np.transpose(x, (1, 0, 2)))      # (k,128,D)->(128,k,D)
    ls_a = i["ls_attn"]
    WeP = i["We"] * ls_a[:, None]                      # fold ls_attn
    pack64 = np.stack([i["ln_r_g"], i["ln_r_b"], i["ln_f_g"], i["ln_f_b"],
                       i["ls_ffn"]])                   # (5, 512)
    pack128 = np.stack([i["ln_s_g"], i["ln_s_b"]])     # (2, 256)
    brow = np.stack([i["bq"], i["be"] * ls_a, i["b1"], i["b2"]])  # (4, 512)
    parts = {
        "codesT": pm(i["receiver_codes"].reshape(R, CODE).T.reshape(2, 128, R)),
        "CqT": pm(i["Cq"].T.reshape(2, 128, ST)),
        "CkT": pm(i["Ck"].T.reshape(2, 128, IN)),
        "CvT": pm(i["Cv"].T.reshape(2, 128, IN)),
        "CeT": pm(i["Ce"].T.reshape(2, 128, ST)),
        "C1T": pm(i["C1"].T.reshape(2, 128, ST)),
        "C2T": pm(i["C2"].T.reshape(2, 128, HID)),
        "WqT": pm(i["Wq"].T.reshape(4, 128, INNER)),
        "Wk": pm(i["Wk"].reshape(4, 128, IN)),
        "WvT": pm(i["Wv"].T.reshape(2, 128, INNER)),
        "W1T": pm(i["W1"].T.reshape(4, 128, HID)),
        "W2T": pm(i["W2"].T.reshape(4, 128, ST)),
    }
    mega = np.concatenate([parts[nm].reshape(128, -1) for nm, _, _ in MEGA],
                          axis=1)
    assert mega.shape == (128, MEGA_F)
    common = {
        "recv": f(i["receiver_states"].reshape(R, ST)),
        "mega": f(mega),
        "WeT8": pm(WeP.T.reshape(8, 64, ST)),
        "pack64": f(np.broadcast_to(pack64[None], (R, 5, ST))),
        "pack128": f(np.broadcast_to(pack128[None], (128, 2, IN))),
        "brow": f(brow[None]),
        "bvexp": f(np.broadcast_to(i["bv"].reshape(8, 64).T[:, :, None],
                                   (64, 8, 64))),
    }
    in_maps = []
    for c in range(N_CORES):
        m = dict(common)
        shard = i["sender_states"][:, c * VC:(c + 1) * VC, :]     # (B, VC, IN)
        m["send"] = pm(shard.reshape(S, IN).reshape(4, 128, IN))
        in_maps.append(m)
    return in_maps


def kernel(**inputs) -> np.ndarray:
    nc = _get_nc()
    in_maps = make_in_maps(inputs)
    res = bass_utils.run_bass_kernel_spmd(nc, in_maps,
                                          core_ids=list(range(N_CORES)))
    return res.results[0]["out"].reshape(B, U, ST).astype(np.float32)


# revision 20
# speedup vs baseline: 1.2804x; 1.0444x over previous
"""Trainium2 Bass kernel for nn_AttentiveReadIn (v2).

Strategy: shard the sender dim V across 8 cores (sequence parallel).
The per-receiver key/value modulation is folded algebraically into the
query / output side so the huge (b,v,u,.) tensors are never
materialized:

  scores(r,h,v) = sum_i [ (q_h @ Wk_h) * scale_k ](r,h,i) * s_ln(v,i)
  ctx(r,h,i)    = sum_v exp(scores)(r,h,v) * s_ln(v,i)
  msg(r,(h,d))  = sum_i ctx(r,h,i) * scale_v(r,i) * Wv((h,d),i)

v2 changes vs v1:
  - all matmul operands in fp16 (validated 6.7e-4 rel err on host sim);
    exp is computed with a -4*ln2 bias (cancels in softmax) so the
    summed exponentials stay in fp16 range.
  - batch-compact score layout: senders only score against their own
    batch's receivers (halves the eT/ctx matmul columns, no masking).
  - the scale_v fold + Wv projection run BEFORE the AllReduce, so the
    collective carries (65, 512) f32 = 133KB (msg partial + sumexp row)
    instead of 528KB of raw ctx.
  - scale_k / scale_v / scale_e are computed directly in transposed
    layout from C^T slices (no tensor-engine transposes for them).
  - ls_attn is folded into We/be on the host; biases enter via K=1
    ones-row matmuls instead of vector adds.

Debug knobs (env): NO_COLL=1 replaces the AllReduce with a local copy;
SIM_GELU_ID=1 swaps gelu for identity; KTEST=1 drops the ones-row
matmuls; KCUT=n truncates the kernel after stage n (bisection).
"""

import os as _osK

import numpy as np

import concourse.bass as bass
import concourse.mybir as mybir
import concourse.tile as tile
from concourse import bacc, bass_utils
from concourse.masks import make_identity

B, U, V = 2, 32, 2048
IN, ST, CODE = 256, 512, 256
H, HD = 8, 64
INNER = H * HD
HID = ST
N_CORES = 8
R = B * U                      # 64 receiver rows
VC = V // N_CORES              # 256 senders per core per batch
S = B * VC                     # 512 sender rows per core
EPS = 1e-5
SHIFT = float(-4.0 * np.log(2.0))   # exp bias; cancels in softmax

F32 = mybir.dt.float32
MMDT = mybir.dt.float16        # matmul operand dtype
NPDT = np.float16
AXIS = mybir.AluOpType

# all matmul-operand weights packed into one (128, k*D) DMA, score-path first
MEGA = [("codesT", 2, 64), ("CqT", 2, 512), ("WqT", 4, 512),
        ("CkT", 2, 256), ("Wk", 4, 256), ("CvT", 2, 256), ("WvT", 2, 512),
        ("CeT", 2, 512), ("C1T", 2, 512), ("C2T", 2, 512),
        ("W1T", 4, 512), ("W2T", 4, 512)]
MEGA_F = sum(k * d for _, k, d in MEGA)


class _Cut(Exception):
    pass


def _build(nc):
    KT1 = _osK.environ.get("KTEST", "0") == "1"
    KCUT = int(_osK.environ.get("KCUT", "0"))
    d = {}
    def din(name, shape, dt=MMDT):
        d[name] = nc.dram_tensor(name, list(shape), dt, kind="ExternalInput")
        return d[name]

    din("send", (128, 4, IN))            # per-core sender shard (part-major)
    din("recv", (R, ST))
    din("mega", (128, MEGA_F))
    din("WeT8", (64, 8, ST))             # ls_attn folded into ST cols
    din("pack64", (R, 5, ST))            # ln_r_g/b, ln_f_g/b, ls_ffn
    din("pack128", (128, 2, IN))         # ln_s_g/b
    din("brow", (1, 4, ST))              # bq, be*ls_attn, b1, b2
    din("bvexp", (64, 8, 64))            # bv as (hd, h, r)
    din("sel4", (4, 4, 64))              # row-select for Z broadcast
    din("sel4", (4, 4, 64))              # row-select for Z broadcast
    out = nc.dram_tensor("out", [R, ST], F32, kind="ExternalOutput")

    from contextlib import ExitStack
    with tile.TileContext(nc) as tc, ExitStack() as es:
        wpool = es.enter_context(tc.tile_pool(name="w", bufs=1))
        apool = es.enter_context(tc.tile_pool(name="a", bufs=1))
        tpool = es.enter_context(tc.tile_pool(name="t", bufs=3))
        ps_g = es.enter_context(tc.tile_pool(name="ps_g", bufs=2, space="PSUM"))
        ps_sc = es.enter_context(tc.tile_pool(name="ps_sc", bufs=2, space="PSUM"))
        ps_z = es.enter_context(tc.tile_pool(name="ps_z", bufs=1, space="PSUM"))
        dpool = es.enter_context(tc.tile_pool(name="dram", bufs=1, space="DRAM"))

        def sb(pool, name, shape, dt=F32, bufs=None):
            return pool.tile(list(shape), dt, tag=name, name=name, bufs=bufs)

        def cut(k):
            if KCUT == k:
                dbg = sb(apool, "dbg", (R, ST))
                nc.vector.memset(dbg[:], 0.0)
                nc.sync.dma_start(out=out.ap(), in_=dbg[:])
                raise _Cut()

        try:
            _kbody(nc, d, out, KT1, sb, cut, wpool, apool, tpool,
                   ps_g, ps_sc, ps_z, dpool)
        except _Cut:
            pass

    nc.compile()
    return nc


def _kbody(nc, d, out, KT1, sb, cut, wpool, apool, tpool,
           ps_g, ps_sc, ps_z, dpool):
    # ---- load everything ----
    def load(name, shape, dt=MMDT):
        t = sb(wpool, name, list(shape), dt)
        nc.sync.dma_start(out=t[:], in_=d[name].ap())
        return t

    send = load("send", (128, 4, IN))
    recv = load("recv", (R, ST))
    mega = sb(wpool, "mega", (128, MEGA_F), MMDT)
    def _megaoff(names):
        o = 0
        for nm, k, dd in MEGA:
            if nm in names:
                o += k * dd
            else:
                break
        return o
    _c1 = _megaoff(("codesT", "CqT"))
    _c2 = _megaoff(("codesT", "CqT", "WqT"))
    _sc_end = _megaoff(("codesT", "CqT", "WqT", "CkT", "Wk"))
    _val_end = _megaoff(("codesT", "CqT", "WqT", "CkT", "Wk", "CvT", "WvT"))
    pack128 = load("pack128", (128, 2, IN))
    pack64 = load("pack64", (R, 5, ST))
    brow = load("brow", (1, 4, ST))
    nc.sync.dma_start(out=mega[:, :_c1], in_=d["mega"].ap()[:, :_c1])
    nc.sync.dma_start(out=mega[:, _c1:_c2], in_=d["mega"].ap()[:, _c1:_c2])
    nc.sync.dma_start(out=mega[:, _c2:_sc_end],
                      in_=d["mega"].ap()[:, _c2:_sc_end])
    nc.sync.dma_start(out=mega[:, _sc_end:_val_end],
                      in_=d["mega"].ap()[:, _sc_end:_val_end])
    WeT8 = load("WeT8", (64, 8, ST))
    bvexp = load("bvexp", (64, 8, 64))
    sel4 = load("sel4", (4, 4, 64))
    sel4 = load("sel4", (4, 4, 64))
    nc.sync.dma_start(out=mega[:, _val_end:],
                      in_=d["mega"].ap()[:, _val_end:])
    _views, _off = {}, 0
    for _nm, _k, _d in MEGA:
        _views[_nm] = mega[:, _off:_off + _k * _d].rearrange(
            "p (k d) -> p k d", k=_k)
        _off += _k * _d
    codesT, CqT, CkT = _views["codesT"], _views["CqT"], _views["CkT"]
    WqT, Wk, CvT, CeT = _views["WqT"], _views["Wk"], _views["CvT"], _views["CeT"]
    WvT, C1T, C2T = _views["WvT"], _views["C1T"], _views["C2T"]
    W1T, W2T = _views["W1T"], _views["W2T"]
    _p64 = ["ln_r_g", "ln_r_b", "ln_f_g", "ln_f_b", "ls_ffn"]
    bc = {nm: pack64[:, j, :] for j, nm in enumerate(_p64)}
    bc["ln_s_g"] = pack128[:, 0, :]
    bc["ln_s_b"] = pack128[:, 1, :]

    epst = sb(wpool, "epst", (128, 1))
    nc.vector.memset(epst[:], EPS)
    ident32 = sb(wpool, "ident32", (128, 128), F32)
    make_identity(nc, ident32[:])
    ident = sb(wpool, "ident", (128, 128), MMDT)
    nc.vector.tensor_copy(out=ident[:], in_=ident32[:])
    onesA = sb(wpool, "onesA", (1, 64), MMDT)
    nc.vector.memset(onesA[:], 1.0)
    ones128 = sb(wpool, "ones128", (128, 1), MMDT)
    nc.vector.memset(ones128[:], 1.0)
    shiftt = sb(wpool, "shiftt", (128, 1))
    nc.vector.memset(shiftt[:], SHIFT)

    def transpose(dst_ps, src_ap):
        p = src_ap.shape[0]
        idt = ident if src_ap.dtype == MMDT else ident32
        nc.tensor.transpose(dst_ps, src_ap, idt[:p, :p])

    # ---- natural scales (R, D) = 1 + codes @ C^T : q, f1, f2 ----
    scales = {}
    for nm, CT, D in [("q", CqT, ST), ("f1", C1T, ST), ("f2", C2T, HID)]:
        p = sb(ps_g, "g", (R, 512))
        for j in range(2):
            nc.tensor.matmul(p[:, :D], codesT[:, j, :], CT[:, j, :],
                             start=(j == 0), stop=(j == 1))
        s = sb(apool, "scale_" + nm, (R, D), MMDT)
        nc.scalar.add(out=s[:], in_=p[:, :D], add=1.0)
        scales[nm] = s

    # ---- transposed scales: skT/svT (i, 2c, r), seT8 (hd, h, r) ----
    skT = sb(apool, "skT", (128, 2, R), MMDT)
    svT = sb(apool, "svT", (128, 2, R), MMDT)
    for CT, dst in [(CkT, skT), (CvT, svT)]:
        for c in range(2):
            p = sb(ps_g, "g", (128, R))
            for j in range(2):
                nc.tensor.matmul(p[:], CT[:, j, c * 128:(c + 1) * 128],
                                 codesT[:, j, :], start=(j == 0), stop=(j == 1))
            nc.scalar.add(out=dst[:, c, :], in_=p[:], add=1.0)
    # f-LN gain/bias folded into scale_f1 (used post-AR)
    sf1g = sb(apool, "sf1g", (R, ST), MMDT)
    nc.vector.tensor_mul(out=sf1g[:], in0=scales["f1"][:], in1=bc["ln_f_g"])
    bf1 = sb(apool, "bf1", (R, ST), MMDT)
    nc.vector.tensor_mul(out=bf1[:], in0=scales["f1"][:], in1=bc["ln_f_b"])
    seT8 = sb(apool, "seT8", (64, H, R), MMDT)
    for ic in range(4):
        p = sb(ps_g, "g", (128, R))
        for j in range(2):
            nc.tensor.matmul(p[:], CeT[:, j, ic * 128:(ic + 1) * 128],
                             codesT[:, j, :], start=(j == 0), stop=(j == 1))
        nc.scalar.add(out=seT8[:, 2 * ic, :], in_=p[:64, :], add=1.0)
        nc.scalar.add(out=seT8[:, 2 * ic + 1, :], in_=p[64:, :], add=1.0)

    cut(1)

    # ---- receiver layernorm + x_q ----
    mvr = sb(apool, "mvr", (R, 2))
    bnr = sb(apool, "bnr", (R, 6))
    nc.vector.bn_stats(out=bnr[:], in_=recv[:])
    nc.vector.bn_aggr(out=mvr[:], in_=bnr[:])
    rstd_r = sb(apool, "rstd_r", (R, 1))
    nc.scalar.activation(out=rstd_r[:], in_=mvr[:, 1:2],
                         func=mybir.ActivationFunctionType.Sqrt, bias=epst[:R])
    nc.vector.reciprocal(out=rstd_r[:], in_=rstd_r[:])
    zr = sb(apool, "zr", (R, ST))
    nc.vector.tensor_scalar(out=zr[:], in0=recv[:], scalar1=mvr[:, 0:1],
                            scalar2=rstd_r[:], op0=AXIS.subtract, op1=AXIS.mult)
    nc.vector.tensor_mul(out=zr[:], in0=zr[:], in1=bc["ln_r_g"])
    nc.vector.tensor_add(out=zr[:], in0=zr[:], in1=bc["ln_r_b"])
    xq = sb(apool, "xq", (R, ST), MMDT)
    nc.vector.tensor_mul(out=xq[:], in0=zr[:], in1=scales["q"][:])

    cut(11)

    # ---- q = xq @ Wq^T + bq (bias via ones-row matmul) ----
    xqT = sb(apool, "xqT", (128, 4, R), MMDT)
    for t in range(4):
        p = sb(ps_g, "gt", (128, 128), MMDT, bufs=1)
        transpose(p[:, :R], xq[:, t * 128:(t + 1) * 128])
        nc.vector.tensor_copy(out=xqT[:, t, :], in_=p[:, :R])

    cut(12)

    qps = sb(ps_g, "g", (R, INNER))
    for t in range(4):
        nc.tensor.matmul(qps[:], xqT[:, t, :], WqT[:, t, :],
                         start=(t == 0), stop=(KT1 and t == 3))
    if not KT1:
        nc.tensor.matmul(qps[:], onesA[:1, :], brow[:, 0, :],
                         start=False, stop=True)
    q_sb = sb(apool, "q_sb", (R, INNER), MMDT)
    nc.vector.tensor_copy(out=q_sb[:], in_=qps[:])

    cut(13)
    qT = sb(apool, "qT", (128, 4, R), MMDT)
    for t in range(4):
        p = sb(ps_g, "gt", (128, 128), MMDT, bufs=1)
        transpose(p[:, :R], q_sb[:, t * 128:(t + 1) * 128])
        nc.vector.tensor_copy(out=qT[:, t, :], in_=p[:, :R])

    cut(14)

    # ---- qkT(i,(h,r)) = [sum_d Wk((h,d),i) qT((h,d),r)] * skT ----
    qkT = sb(apool, "qkT", (128, 2, H, R), MMDT)
    for c in range(2):
        for h in range(H):
            t, o = h // 2, (h % 2) * 64
            p = sb(ps_g, "gqk", (128, R), bufs=2)
            nc.tensor.matmul(p[:],
                             Wk[o:o + 64, t, c * 128:(c + 1) * 128],
                             qT[o:o + 64, t, :], start=True, stop=True)
            nc.vector.tensor_mul(out=qkT[:, c, h, :], in0=p[:],
                                 in1=skT[:, c, :])

    cut(15)

    cut(2)

    # ---- sender layernorm (natural) ----
    slna = sb(apool, "slna", (128, 4, IN), MMDT)
    for t in range(4):
        bns = sb(tpool, "bns", (128, 6))
        mvs = sb(tpool, "mvs", (128, 2))
        nc.vector.bn_stats(out=bns[:], in_=send[:, t, :])
        nc.vector.bn_aggr(out=mvs[:], in_=bns[:])
        rstd = sb(tpool, "rstd_s", (128, 1))
        nc.scalar.activation(out=rstd[:], in_=mvs[:, 1:2],
                             func=mybir.ActivationFunctionType.Sqrt, bias=epst[:])
        nc.vector.reciprocal(out=rstd[:], in_=rstd[:])
        zs = sb(tpool, "zs", (128, IN))
        nc.vector.tensor_scalar(out=zs[:], in0=send[:, t, :],
                                scalar1=mvs[:, 0:1], scalar2=rstd[:],
                                op0=AXIS.subtract, op1=AXIS.mult)
        nc.vector.tensor_mul(out=zs[:], in0=zs[:], in1=bc["ln_s_g"])
        nc.vector.tensor_add(out=slna[:, t, :], in0=zs[:], in1=bc["ln_s_b"])

    # ---- s_ln^T (i, s) ----
    slnT = sb(apool, "slnT", (128, 2, S), MMDT)
    for c in range(2):
        for t in range(4):
            p = sb(ps_g, "gt", (128, 128), MMDT, bufs=1)
            transpose(p[:], slna[:, t, c * 128:(c + 1) * 128])
            nc.vector.tensor_copy(out=slnT[:, c, t * 128:(t + 1) * 128],
                                  in_=p[:])

    # ---- scoresT -> exp (batch-compact: tile t scores batch t//2) ----
    eT = sb(apool, "eT", (128, 4, H * U), MMDT)
    for t in range(4):
        b = t // 2
        p = sb(ps_sc, "ps_scores", (128, H * U))
        for c in range(2):
            nc.tensor.matmul(
                p[:], slnT[:, c, t * 128:(t + 1) * 128],
                qkT[:, c, :, b * U:(b + 1) * U],
                start=(c == 0), stop=(c == 1))
        nc.scalar.activation(out=eT[:, t, :], in_=p[:],
                             func=mybir.ActivationFunctionType.Exp,
                             scale=float(1.0 / np.sqrt(HD)), bias=shiftt[:])

    # keep the sqrt table resident for the post-AR layernorm: touch Sqrt
    # after the last Exp so no table load lands on the tail critical path
    tdum = sb(apool, "tdum", (1, 1))
    nc.scalar.activation(out=tdum[:], in_=eT[:1, 3, :1],
                         func=mybir.ActivationFunctionType.Sqrt)

    cut(3)

    # ---- AR buffer: rows 0-63 msg partial (hd,(h,b,u)), row 64 sumexp ----
    armsg = sb(apool, "armsg", (65, H, B, U), MMDT)
    ar_in = dpool.tile([65, 512], MMDT, tag="ar_in", name="ar_in")
    ar_out = dpool.tile([65, 512], MMDT, tag="ar_out", name="ar_out")

    # Z row: zps(1, (b,h,u)) = colsum of eT
    if not KT1:
        for b in range(2):
            zps = sb(ps_z, "ps_z", (1, 256))
            for k, t in enumerate((2 * b, 2 * b + 1)):
                nc.tensor.matmul(zps[:], ones128[:],
                                 eT[:, t, :], start=(k == 0), stop=(k == 1))
            nc.vector.tensor_copy(
                out=armsg[64:65, :, b, :],
                in_=zps[:].rearrange("p (h u) -> p h u", h=8))
    else:
        nc.vector.memset(armsg[64:65, :, :, :], 1.0)

    # ---- ctxT(i, (b,h,u)) directly: slna^T stationary vs eT moving ----
    ctxTs = sb(apool, "ctxTs", (128, 2, B, H, U), MMDT)
    for c in range(2):
        for b in range(2):
            p = sb(ps_sc, "ps_scores", (128, H * U))
            for k, t in enumerate((2 * b, 2 * b + 1)):
                nc.tensor.matmul(p[:], slna[:, t, c * 128:(c + 1) * 128],
                                 eT[:, t, :], start=(k == 0), stop=(k == 1))
            nc.vector.tensor_mul(
                out=ctxTs[:, c, b, :, :],
                in0=p[:].rearrange("p (h u) -> p h u", h=H),
                in1=svT[:, c, b * U:(b + 1) * U].unsqueeze(1)
                    .broadcast_to([128, H, U]))

    # ---- msg partial: per head, Wv^T contraction ----
    for h in range(H):
        p = sb(ps_g, "g", (64, R))
        for c in range(2):
            nc.tensor.matmul(
                p[:], WvT[:, c, h * 64:(h + 1) * 64],
                ctxTs[:, c, :, h, :],
                start=(c == 0), stop=(c == 1))
        nc.vector.tensor_copy(out=armsg[:64, h, :, :]
                              .rearrange("p b u -> p (b u)"), in_=p[:])

    cut(4)

    nc.sync.dma_start(out=ar_in[:],
                      in_=armsg[:].rearrange("p h b u -> p (h b u)"))
    if _osK.environ.get("NO_COLL") == "1":
        nc.sync.dma_start(out=ar_out[:], in_=ar_in[:])
    else:
        nc.gpsimd.collective_compute(
            "AllReduce", AXIS.add,
            replica_groups=[list(range(N_CORES))],
            ins=[ar_in.opt()], outs=[ar_out.opt()])

    # ---- post-AR: normalize, +bv, *scale_e, exit proj ----
    csall = sb(apool, "csall", (64, 512), MMDT)
    nc.sync.dma_start(out=csall[:], in_=ar_out[:64, :])
    zsp = sb(apool, "zsp", (4, 128), MMDT)
    nc.sync.dma_start(out=zsp[:],
                      in_=ar_out[64:65, :].rearrange("p (q x) -> (p q) x", q=4))
    zrec = sb(apool, "zrec", (4, 128))
    nc.vector.reciprocal(out=zrec[:], in_=zsp[:])
    zrec16 = sb(apool, "zrec16", (4, 128), MMDT)
    nc.vector.tensor_copy(out=zrec16[:], in_=zrec[:])
    msgn = sb(apool, "msgn", (64, 512))
    if not KT1:
        for j in range(4):
            zbps = sb(ps_g, "gqk", (64, 128), bufs=2)
            nc.tensor.matmul(zbps[:], sel4[:, j, :], zrec16[:],
                             start=True, stop=True)
            nc.vector.tensor_mul(out=msgn[:, j * 128:(j + 1) * 128],
                                 in0=csall[:64, j * 128:(j + 1) * 128],
                                 in1=zbps[:])
    else:
        nc.vector.tensor_copy(out=msgn[:], in_=csall[:64, :])
    nc.vector.tensor_add(out=msgn[:], in0=msgn[:],
                         in1=bvexp[:].rearrange("p h u -> p (h u)"))
    y8 = sb(apool, "y8", (64, H, R), MMDT)
    nc.vector.tensor_mul(out=y8[:].rearrange("p h u -> p (h u)"),
                         in0=msgn[:],
                         in1=seT8[:].rearrange("p h u -> p (h u)"))
    xps = sb(ps_z, "ps_z", (R, ST), bufs=1)
    for h in range(H):
        nc.tensor.matmul(xps[:], y8[:, h, :], WeT8[:, h, :],
                         start=(h == 0), stop=(KT1 and h == H - 1))
    if not KT1:
        nc.tensor.matmul(xps[:], onesA[:1, :], brow[:, 1, :],
                         start=False, stop=True)
    x_att = xps

    cut(5)

    # ---- FFN ----
    bnf = sb(apool, "bnf", (R, 6))
    mvf = sb(apool, "mvf", (R, 2))
    nc.vector.bn_stats(out=bnf[:], in_=x_att[:])
    nc.vector.bn_aggr(out=mvf[:], in_=bnf[:])
    rstd_f = sb(apool, "rstd_f", (R, 1))
    nc.scalar.activation(out=rstd_f[:], in_=mvf[:, 1:2],
                         func=mybir.ActivationFunctionType.Sqrt, bias=epst[:R])
    nc.vector.reciprocal(out=rstd_f[:], in_=rstd_f[:])
    zf = sb(apool, "zf", (R, ST))
    nc.vector.tensor_scalar(out=zf[:], in0=x_att[:], scalar1=mvf[:, 0:1],
                            scalar2=rstd_f[:], op0=AXIS.subtract, op1=AXIS.mult)
    x1 = sb(apool, "x1", (R, ST), MMDT)
    nc.vector.tensor_mul(out=x1[:], in0=zf[:], in1=sf1g[:])
    nc.vector.tensor_add(out=x1[:], in0=x1[:], in1=bf1[:])
    x1T = sb(apool, "x1T", (128, 4, R), MMDT)
    for t in range(4):
        p = sb(ps_g, "gt", (128, 128), MMDT, bufs=1)
        transpose(p[:, :R], x1[:, t * 128:(t + 1) * 128])
        nc.vector.tensor_copy(out=x1T[:, t, :], in_=p[:, :R])
    h1ps = sb(ps_g, "g", (R, HID))
    for t in range(4):
        nc.tensor.matmul(h1ps[:], x1T[:, t, :], W1T[:, t, :],
                         start=(t == 0), stop=(KT1 and t == 3))
    if not KT1:
        nc.tensor.matmul(h1ps[:], onesA[:1, :], brow[:, 2, :],
                         start=False, stop=True)
    h1g = sb(apool, "h1g", (R, HID), MMDT)
    _gelu = (mybir.ActivationFunctionType.Identity
             if _osK.environ.get("SIM_GELU_ID") == "1"
             else mybir.ActivationFunctionType.Gelu)
    nc.scalar.activation(out=h1g[:], in_=h1ps[:], func=_gelu)
    h1s = sb(apool, "h1s", (R, HID), MMDT)
    nc.vector.tensor_mul(out=h1s[:], in0=h1g[:], in1=scales["f2"][:])
    h1sT = sb(apool, "h1sT", (128, 4, R), MMDT)
    for t in range(4):
        p = sb(ps_g, "gt", (128, 128), MMDT, bufs=1)
        transpose(p[:, :R], h1s[:, t * 128:(t + 1) * 128])
        nc.vector.tensor_copy(out=h1sT[:, t, :], in_=p[:, :R])
    h2ps = sb(ps_g, "g", (R, ST))
    for t in range(4):
        nc.tensor.matmul(h2ps[:], h1sT[:, t, :], W2T[:, t, :],
                         start=(t == 0), stop=(KT1 and t == 3))
    if not KT1:
        nc.tensor.matmul(h2ps[:], onesA[:1, :], brow[:, 3, :],
                         start=False, stop=True)
    o_sb = sb(apool, "o_sb", (R, ST))
    nc.vector.tensor_mul(out=o_sb[:], in0=h2ps[:], in1=bc["ls_ffn"])
    nc.vector.tensor_add(out=o_sb[:], in0=o_sb[:], in1=x_att[:])
    nc.sync.dma_start(out=out.ap(), in_=o_sb[:])


_NC_CACHE = None


def _get_nc():
    global _NC_CACHE
    if _NC_CACHE is None:
        nc = bacc.Bacc("TRN2", target_bir_lowering=False, debug=False,
                       num_devices=N_CORES)
        _NC_CACHE = _build(nc)
    return _NC_CACHE


def make_in_maps(inputs):
    f = lambda x: np.ascontiguousarray(np.asarray(x, np.float32), dtype=NPDT)
    i = {k: np.asarray(v, np.float32) for k, v in inputs.items()}
    pm = lambda x: f(np.transpose(x, (1, 0, 2)))      # (k,128,D)->(128,k,D)
    ls_a = i["ls_attn"]
    WeP = i["We"] * ls_a[:, None]                      # fold ls_attn
    pack64 = np.stack([i["ln_r_g"], i["ln_r_b"], i["ln_f_g"], i["ln_f_b"],
                       i["ls_ffn"]])                   # (5, 512)
    pack128 = np.stack([i["ln_s_g"], i["ln_s_b"]])     # (2, 256)
    brow = np.stack([i["bq"], i["be"] * ls_a, i["b1"], i["b2"]])  # (4, 512)
    parts = {
        "codesT": pm(i["receiver_codes"].reshape(R, CODE).T.reshape(2, 128, R)),
        "CqT": pm(i["Cq"].T.reshape(2, 128, ST)),
        "CkT": pm(i["Ck"].T.reshape(2, 128, IN)),
        "CvT": pm(i["Cv"].T.reshape(2, 128, IN)),
        "CeT": pm(i["Ce"].T.reshape(2, 128, ST)),
        "C1T": pm(i["C1"].T.reshape(2, 128, ST)),
        "C2T": pm(i["C2"].T.reshape(2, 128, HID)),
        "WqT": pm(i["Wq"].T.reshape(4, 128, INNER)),
        "Wk": pm(i["Wk"].reshape(4, 128, IN)),
        "WvT": pm(i["Wv"].T.reshape(2, 128, INNER)),
        "W1T": pm(i["W1"].T.reshape(4, 128, HID)),
        "W2T": pm(i["W2"].T.reshape(4, 128, ST)),
    }
    mega = np.concatenate([parts[nm].reshape(128, -1) for nm, _, _ in MEGA],
                          axis=1)
    assert mega.shape == (128, MEGA_F)
    common = {
        "recv": f(i["receiver_states"].reshape(R, ST)),
        "mega": f(mega),
        "WeT8": pm(WeP.T.reshape(8, 64, ST)),
        "pack64": f(np.broadcast_to(pack64[None], (R, 5, ST))),
        "pack128": f(np.broadcast_to(pack128[None], (128, 2, IN))),
        "brow": f(brow[None]),
        "bvexp": f(np.broadcast_to(i["bv"].reshape(8, 64).T[:, :, None],
                                   (64, 8, 64))),
        "sel4": f(np.eye(4)[:, :, None] * np.ones((1, 1, 64))),
        "sel4": f(np.eye(4)[:, :, None] * np.ones((1, 1, 64))),
    }
    in_maps = []
    for c in range(N_CORES):
        m = dict(common)
        shard = i["sender_states"][:, c * VC:(c + 1) * VC, :]     # (B, VC, IN)
        m["send"] = pm(shard.reshape(S, IN).reshape(4, 128, IN))
        in_maps.append(m)
    return in_maps


def kernel(**inputs) -> np.ndarray:
    nc = _get_nc()
    in_maps = make_in_maps(inputs)
    res = bass_utils.run_bass_kernel_spmd(nc, in_maps,
                                          core_ids=list(range(N_CORES)))
    return res.results[0]["out"].reshape(B, U, ST).astype(np.float32)


# revision 21
# speedup vs baseline: 1.5275x; 1.1930x over previous
"""Trainium2 Bass kernel for nn_AttentiveReadIn (v2).

Strategy: shard the sender dim V across 8 cores (sequence parallel).
The per-receiver key/value modulation is folded algebraically into the
query / output side so the huge (b,v,u,.) tensors are never
materialized:

  scores(r,h,v) = sum_i [ (q_h @ Wk_h) * scale_k ](r,h,i) * s_ln(v,i)
  ctx(r,h,i)    = sum_v exp(scores)(r,h,v) * s_ln(v,i)
  msg(r,(h,d))  = sum_i ctx(r,h,i) * scale_v(r,i) * Wv((h,d),i)

v2 changes vs v1:
  - all matmul operands in fp16 (validated 6.7e-4 rel err on host sim);
    exp is computed with a -4*ln2 bias (cancels in softmax) so the
    summed exponentials stay in fp16 range.
  - batch-compact score layout: senders only score against their own
    batch's receivers (halves the eT/ctx matmul columns, no masking).
  - the scale_v fold + Wv projection run BEFORE the AllReduce, so the
    collective carries (65, 512) f32 = 133KB (msg partial + sumexp row)
    instead of 528KB of raw ctx.
  - scale_k / scale_v / scale_e are computed directly in transposed
    layout from C^T slices (no tensor-engine transposes for them).
  - ls_attn is folded into We/be on the host; biases enter via K=1
    ones-row matmuls instead of vector adds.

Debug knobs (env): NO_COLL=1 replaces the AllReduce with a local copy;
SIM_GELU_ID=1 swaps gelu for identity; KTEST=1 drops the ones-row
matmuls; KCUT=n truncates the kernel after stage n (bisection).
"""

import os as _osK

import numpy as np

import concourse.bass as bass
import concourse.mybir as mybir
import concourse.tile as tile
from concourse import bacc, bass_utils
from concourse.masks import make_identity

B, U, V = 2, 32, 2048
IN, ST, CODE = 256, 512, 256
H, HD = 8, 64
INNER = H * HD
HID = ST
N_CORES = 8
R = B * U                      # 64 receiver rows
VC = V // N_CORES              # 256 senders per core per batch
S = B * VC                     # 512 sender rows per core
EPS = 1e-5
SHIFT = float(-4.0 * np.log(2.0))   # exp bias; cancels in softmax

F32 = mybir.dt.float32
MMDT = mybir.dt.float16        # matmul operand dtype
NPDT = np.float16
AXIS = mybir.AluOpType

# all matmul-operand weights packed into one (128, k*D) DMA, score-path first
MEGA = [("codesT", 2, 64), ("CqT", 2, 512), ("WqT", 4, 512),
        ("CkT", 2, 256), ("Wk", 4, 256), ("CvT", 2, 256), ("WvT", 2, 512),
        ("CeT", 2, 512), ("C1T", 2, 512), ("C2T", 2, 512),
        ("W1T", 4, 512), ("W2T", 4, 512)]
MEGA_F = sum(k * d for _, k, d in MEGA)


class _Cut(Exception):
    pass


def _build(nc):
    KT1 = _osK.environ.get("KTEST", "0") == "1"
    KCUT = int(_osK.environ.get("KCUT", "0"))
    d = {}
    def din(name, shape, dt=MMDT):
        d[name] = nc.dram_tensor(name, list(shape), dt, kind="ExternalInput")
        return d[name]

    din("send", (128, 4, IN))            # per-core sender shard (part-major)
    din("recv", (R, ST))
    din("mega", (128, MEGA_F))
    din("WeT8", (64, 8, ST))             # ls_attn folded into ST cols
    din("pack64", (R, 5, ST))            # ln_r_g/b, ln_f_g/b, ls_ffn
    din("pack128", (128, 2, IN))         # ln_s_g/b
    din("brow", (1, 4, ST))              # bq, be*ls_attn, b1, b2
    din("bvexp", (64, 8, 64))            # bv as (hd, h, r)
    din("sel4", (4, 4, 64))              # row-select for Z broadcast
    din("sel4", (4, 4, 64))              # row-select for Z broadcast
    out = nc.dram_tensor("out", [R, ST], F32, kind="ExternalOutput")

    from contextlib import ExitStack
    with tile.TileContext(nc) as tc, ExitStack() as es:
        wpool = es.enter_context(tc.tile_pool(name="w", bufs=1))
        apool = es.enter_context(tc.tile_pool(name="a", bufs=1))
        tpool = es.enter_context(tc.tile_pool(name="t", bufs=3))
        ps_g = es.enter_context(tc.tile_pool(name="ps_g", bufs=2, space="PSUM"))
        ps_sc = es.enter_context(tc.tile_pool(name="ps_sc", bufs=2, space="PSUM"))
        ps_z = es.enter_context(tc.tile_pool(name="ps_z", bufs=1, space="PSUM"))
        dpool = es.enter_context(tc.tile_pool(name="dram", bufs=1, space="DRAM"))

        def sb(pool, name, shape, dt=F32, bufs=None):
            return pool.tile(list(shape), dt, tag=name, name=name, bufs=bufs)

        def cut(k):
            if KCUT == k:
                dbg = sb(apool, "dbg", (R, ST))
                nc.vector.memset(dbg[:], 0.0)
                nc.sync.dma_start(out=out.ap(), in_=dbg[:])
                raise _Cut()

        try:
            _kbody(nc, d, out, KT1, sb, cut, wpool, apool, tpool,
                   ps_g, ps_sc, ps_z, dpool)
        except _Cut:
            pass

    nc.compile()
    return nc


def _kbody(nc, d, out, KT1, sb, cut, wpool, apool, tpool,
           ps_g, ps_sc, ps_z, dpool):
    # ---- load everything ----
    def load(name, shape, dt=MMDT):
        t = sb(wpool, name, list(shape), dt)
        nc.sync.dma_start(out=t[:], in_=d[name].ap())
        return t

    send = load("send", (128, 4, IN))
    recv = load("recv", (R, ST))
    mega = sb(wpool, "mega", (128, MEGA_F), MMDT)
    def _megaoff(names):
        o = 0
        for nm, k, dd in MEGA:
            if nm in names:
                o += k * dd
            else:
                break
        return o
    _c1 = _megaoff(("codesT", "CqT"))
    _c2 = _megaoff(("codesT", "CqT", "WqT"))
    _sc_end = _megaoff(("codesT", "CqT", "WqT", "CkT", "Wk"))
    _val_end = _megaoff(("codesT", "CqT", "WqT", "CkT", "Wk", "CvT", "WvT"))
    pack128 = load("pack128", (128, 2, IN))
    pack64 = load("pack64", (R, 5, ST))
    brow = load("brow", (1, 4, ST))
    nc.sync.dma_start(out=mega[:, :_c1], in_=d["mega"].ap()[:, :_c1])
    nc.sync.dma_start(out=mega[:, _c1:_c2], in_=d["mega"].ap()[:, _c1:_c2])
    nc.sync.dma_start(out=mega[:, _c2:_sc_end],
                      in_=d["mega"].ap()[:, _c2:_sc_end])
    nc.sync.dma_start(out=mega[:, _sc_end:_val_end],
                      in_=d["mega"].ap()[:, _sc_end:_val_end])
    WeT8 = load("WeT8", (64, 8, ST))
    bvexp = load("bvexp", (64, 8, 64))
    sel4 = load("sel4", (4, 4, 64))
    sel4 = load("sel4", (4, 4, 64))
    nc.sync.dma_start(out=mega[:, _val_end:],
                      in_=d["mega"].ap()[:, _val_end:])
    _views, _off = {}, 0
    for _nm, _k, _d in MEGA:
        _views[_nm] = mega[:, _off:_off + _k * _d].rearrange(
            "p (k d) -> p k d", k=_k)
        _off += _k * _d
    codesT, CqT, CkT = _views["codesT"], _views["CqT"], _views["CkT"]
    WqT, Wk, CvT, CeT = _views["WqT"], _views["Wk"], _views["CvT"], _views["CeT"]
    WvT, C1T, C2T = _views["WvT"], _views["C1T"], _views["C2T"]
    W1T, W2T = _views["W1T"], _views["W2T"]
    _p64 = ["ln_r_g", "ln_r_b", "ln_f_g", "ln_f_b", "ls_ffn"]
    bc = {nm: pack64[:, j, :] for j, nm in enumerate(_p64)}
    bc["ln_s_g"] = pack128[:, 0, :]
    bc["ln_s_b"] = pack128[:, 1, :]

    epst = sb(wpool, "epst", (128, 1))
    nc.vector.memset(epst[:], EPS)
    ident32 = sb(wpool, "ident32", (128, 128), F32)
    make_identity(nc, ident32[:])
    ident = sb(wpool, "ident", (128, 128), MMDT)
    nc.vector.tensor_copy(out=ident[:], in_=ident32[:])
    onesA = sb(wpool, "onesA", (1, 64), MMDT)
    nc.vector.memset(onesA[:], 1.0)
    ones128 = sb(wpool, "ones128", (128, 1), MMDT)
    nc.vector.memset(ones128[:], 1.0)
    shiftt = sb(wpool, "shiftt", (128, 1))
    nc.vector.memset(shiftt[:], SHIFT)

    def transpose(dst_ps, src_ap):
        p = src_ap.shape[0]
        idt = ident if src_ap.dtype == MMDT else ident32
        nc.tensor.transpose(dst_ps, src_ap, idt[:p, :p])

    # ---- natural scales (R, D) = 1 + codes @ C^T : q, f1, f2 ----
    scales = {}
    for nm, CT, D in [("q", CqT, ST), ("f1", C1T, ST), ("f2", C2T, HID)]:
        p = sb(ps_g, "g", (R, 512))
        for j in range(2):
            nc.tensor.matmul(p[:, :D], codesT[:, j, :], CT[:, j, :],
                             start=(j == 0), stop=(j == 1))
        s = sb(apool, "scale_" + nm, (R, D), MMDT)
        nc.scalar.add(out=s[:], in_=p[:, :D], add=1.0)
        scales[nm] = s

    # ---- transposed scales: skT/svT (i, 2c, r), seT8 (hd, h, r) ----
    skT = sb(apool, "skT", (128, 2, R), MMDT)
    svT = sb(apool, "svT", (128, 2, R), MMDT)
    for CT, dst in [(CkT, skT), (CvT, svT)]:
        for c in range(2):
            p = sb(ps_g, "g", (128, R))
            for j in range(2):
                nc.tensor.matmul(p[:], CT[:, j, c * 128:(c + 1) * 128],
                                 codesT[:, j, :], start=(j == 0), stop=(j == 1))
            nc.scalar.add(out=dst[:, c, :], in_=p[:], add=1.0)
    # f-LN gain/bias folded into scale_f1 (used post-AR)
    sf1g = sb(apool, "sf1g", (R, ST), MMDT)
    nc.vector.tensor_mul(out=sf1g[:], in0=scales["f1"][:], in1=bc["ln_f_g"])
    bf1 = sb(apool, "bf1", (R, ST), MMDT)
    nc.vector.tensor_mul(out=bf1[:], in0=scales["f1"][:], in1=bc["ln_f_b"])
    seT8 = sb(apool, "seT8", (64, H, R), MMDT)
    for ic in range(4):
        p = sb(ps_g, "g", (128, R))
        for j in range(2):
            nc.tensor.matmul(p[:], CeT[:, j, ic * 128:(ic + 1) * 128],
                             codesT[:, j, :], start=(j == 0), stop=(j == 1))
        nc.scalar.add(out=seT8[:, 2 * ic, :], in_=p[:64, :], add=1.0)
        nc.scalar.add(out=seT8[:, 2 * ic + 1, :], in_=p[64:, :], add=1.0)

    cut(1)

    # ---- receiver layernorm + x_q ----
    mvr = sb(apool, "mvr", (R, 2))
    bnr = sb(apool, "bnr", (R, 6))
    nc.vector.bn_stats(out=bnr[:], in_=recv[:])
    nc.vector.bn_aggr(out=mvr[:], in_=bnr[:])
    rstd_r = sb(apool, "rstd_r", (R, 1))
    nc.scalar.activation(out=rstd_r[:], in_=mvr[:, 1:2],
                         func=mybir.ActivationFunctionType.Sqrt, bias=epst[:R])
    nc.vector.reciprocal(out=rstd_r[:], in_=rstd_r[:])
    zr = sb(apool, "zr", (R, ST))
    nc.vector.tensor_scalar(out=zr[:], in0=recv[:], scalar1=mvr[:, 0:1],
                            scalar2=rstd_r[:], op0=AXIS.subtract, op1=AXIS.mult)
    nc.vector.tensor_mul(out=zr[:], in0=zr[:], in1=bc["ln_r_g"])
    nc.vector.tensor_add(out=zr[:], in0=zr[:], in1=bc["ln_r_b"])
    xq = sb(apool, "xq", (R, ST), MMDT)
    nc.vector.tensor_mul(out=xq[:], in0=zr[:], in1=scales["q"][:])

    cut(11)

    # ---- q = xq @ Wq^T + bq (bias via ones-row matmul) ----
    xqT = sb(apool, "xqT", (128, 4, R), MMDT)
    for t in range(4):
        p = sb(ps_g, "gt", (128, 128), MMDT, bufs=1)
        transpose(p[:, :R], xq[:, t * 128:(t + 1) * 128])
        nc.any.tensor_copy(out=xqT[:, t, :], in_=p[:, :R])

    cut(12)

    qps = sb(ps_g, "g", (R, INNER))
    for t in range(4):
        nc.tensor.matmul(qps[:], xqT[:, t, :], WqT[:, t, :],
                         start=(t == 0), stop=(KT1 and t == 3))
    if not KT1:
        nc.tensor.matmul(qps[:], onesA[:1, :], brow[:, 0, :],
                         start=False, stop=True)
    q_sb = sb(apool, "q_sb", (R, INNER), MMDT)
    nc.any.tensor_copy(out=q_sb[:], in_=qps[:])

    cut(13)
    qT = sb(apool, "qT", (128, 4, R), MMDT)
    for t in range(4):
        p = sb(ps_g, "gt", (128, 128), MMDT, bufs=1)
        transpose(p[:, :R], q_sb[:, t * 128:(t + 1) * 128])
        nc.any.tensor_copy(out=qT[:, t, :], in_=p[:, :R])

    cut(14)

    # ---- qkT(i,(h,r)) = [sum_d Wk((h,d),i) qT((h,d),r)] * skT ----
    qkT = sb(apool, "qkT", (128, 2, H, R), MMDT)
    for c in range(2):
        for h in range(H):
            t, o = h // 2, (h % 2) * 64
            p = sb(ps_g, "gqk", (128, R), bufs=2)
            nc.tensor.matmul(p[:],
                             Wk[o:o + 64, t, c * 128:(c + 1) * 128],
                             qT[o:o + 64, t, :], start=True, stop=True)
            nc.vector.tensor_mul(out=qkT[:, c, h, :], in0=p[:],
                                 in1=skT[:, c, :])

    cut(15)

    cut(2)

    # ---- sender layernorm (natural) ----
    slna = sb(apool, "slna", (128, 4, IN), MMDT)
    for t in range(4):
        bns = sb(tpool, "bns", (128, 6))
        mvs = sb(tpool, "mvs", (128, 2))
        nc.vector.bn_stats(out=bns[:], in_=send[:, t, :])
        nc.vector.bn_aggr(out=mvs[:], in_=bns[:])
        rstd = sb(tpool, "rstd_s", (128, 1))
        nc.scalar.activation(out=rstd[:], in_=mvs[:, 1:2],
                             func=mybir.ActivationFunctionType.Sqrt, bias=epst[:])
        nc.vector.reciprocal(out=rstd[:], in_=rstd[:])
        zs = sb(tpool, "zs", (128, IN))
        nc.vector.tensor_scalar(out=zs[:], in0=send[:, t, :],
                                scalar1=mvs[:, 0:1], scalar2=rstd[:],
                                op0=AXIS.subtract, op1=AXIS.mult)
        nc.vector.tensor_mul(out=zs[:], in0=zs[:], in1=bc["ln_s_g"])
        nc.vector.tensor_add(out=slna[:, t, :], in0=zs[:], in1=bc["ln_s_b"])

    # ---- s_ln^T (i, s) via DMA transpose (keeps PE/DVE free) ----
    slnT = sb(apool, "slnT", (128, 2, S), MMDT)
    for c in range(2):
        for t in range(4):
            nc.sync.dma_start(out=slnT[:, c, t * 128:(t + 1) * 128],
                              in_=slna[:, t, c * 128:(c + 1) * 128],
                              transpose=True)

    # ---- scoresT -> exp (batch-compact: tile t scores batch t//2) ----
    eT = sb(apool, "eT", (128, 4, H * U), MMDT)
    for t in range(4):
        b = t // 2
        p = sb(ps_sc, "ps_scores", (128, H * U))
        for c in range(2):
            nc.tensor.matmul(
                p[:], slnT[:, c, t * 128:(t + 1) * 128],
                qkT[:, c, :, b * U:(b + 1) * U],
                start=(c == 0), stop=(c == 1))
        nc.scalar.activation(out=eT[:, t, :], in_=p[:],
                             func=mybir.ActivationFunctionType.Exp,
                             scale=float(1.0 / np.sqrt(HD)), bias=shiftt[:])

    # keep the sqrt table resident for the post-AR layernorm: touch Sqrt
    # after the last Exp so no table load lands on the tail critical path
    tdum = sb(apool, "tdum", (1, 1))
    nc.scalar.activation(out=tdum[:], in_=eT[:1, 3, :1],
                         func=mybir.ActivationFunctionType.Sqrt)

    cut(3)

    # ---- AR buffer: rows 0-63 msg partial (hd,(h,b,u)), row 64 sumexp ----
    armsg = sb(apool, "armsg", (65, H, B, U), MMDT)
    ar_in = dpool.tile([65, 512], MMDT, tag="ar_in", name="ar_in")
    ar_out = dpool.tile([65, 512], MMDT, tag="ar_out", name="ar_out")

    # Z row: zps(1, (b,h,u)) = colsum of eT
    if not KT1:
        for b in range(2):
            zps = sb(ps_z, "ps_z", (1, 256))
            for k, t in enumerate((2 * b, 2 * b + 1)):
                nc.tensor.matmul(zps[:], ones128[:],
                                 eT[:, t, :], start=(k == 0), stop=(k == 1))
            nc.vector.tensor_copy(
                out=armsg[64:65, :, b, :],
                in_=zps[:].rearrange("p (h u) -> p h u", h=8))
    else:
        nc.vector.memset(armsg[64:65, :, :, :], 1.0)

    # ---- ctxT(i, (b,h,u)) directly: slna^T stationary vs eT moving ----
    ctxTs = sb(apool, "ctxTs", (128, 2, B, H, U), MMDT)
    for c in range(2):
        for b in range(2):
            p = sb(ps_sc, "ps_scores", (128, H * U))
            for k, t in enumerate((2 * b, 2 * b + 1)):
                nc.tensor.matmul(p[:], slna[:, t, c * 128:(c + 1) * 128],
                                 eT[:, t, :], start=(k == 0), stop=(k == 1))
            nc.vector.tensor_mul(
                out=ctxTs[:, c, b, :, :],
                in0=p[:].rearrange("p (h u) -> p h u", h=H),
                in1=svT[:, c, b * U:(b + 1) * U].unsqueeze(1)
                    .broadcast_to([128, H, U]))

    # ---- msg partial: per head, Wv^T contraction ----
    for h in range(H):
        p = sb(ps_g, "g", (64, R))
        for c in range(2):
            nc.tensor.matmul(
                p[:], WvT[:, c, h * 64:(h + 1) * 64],
                ctxTs[:, c, :, h, :],
                start=(c == 0), stop=(c == 1))
        nc.any.tensor_copy(out=armsg[:64, h, :, :]
                              .rearrange("p b u -> p (b u)"), in_=p[:])

    cut(4)

    nc.sync.dma_start(out=ar_in[:],
                      in_=armsg[:].rearrange("p h b u -> p (h b u)"))
    if _osK.environ.get("NO_COLL") == "1":
        nc.sync.dma_start(out=ar_out[:], in_=ar_in[:])
    else:
        nc.gpsimd.collective_compute(
            "AllReduce", AXIS.add,
            replica_groups=[list(range(N_CORES))],
            ins=[ar_in.opt()], outs=[ar_out.opt()])

    # ---- post-AR: normalize, +bv, *scale_e, exit proj ----
    csall = sb(apool, "csall", (64, 512), MMDT)
    nc.sync.dma_start(out=csall[:], in_=ar_out[:64, :])
    zsp = sb(apool, "zsp", (4, 128), MMDT)
    nc.sync.dma_start(out=zsp[:],
                      in_=ar_out[64:65, :].rearrange("p (q x) -> (p q) x", q=4))
    zrec = sb(apool, "zrec", (4, 128))
    nc.vector.reciprocal(out=zrec[:], in_=zsp[:])
    zrec16 = sb(apool, "zrec16", (4, 128), MMDT)
    nc.vector.tensor_copy(out=zrec16[:], in_=zrec[:])
    msgn = sb(apool, "msgn", (64, 512))
    if not KT1:
        for j in range(4):
            zbps = sb(ps_g, "gqk", (64, 128), bufs=2)
            nc.tensor.matmul(zbps[:], sel4[:, j, :], zrec16[:],
                             start=True, stop=True)
            nc.vector.tensor_mul(out=msgn[:, j * 128:(j + 1) * 128],
                                 in0=csall[:64, j * 128:(j + 1) * 128],
                                 in1=zbps[:])
    else:
        nc.vector.tensor_copy(out=msgn[:], in_=csall[:64, :])
    nc.vector.tensor_add(out=msgn[:], in0=msgn[:],
                         in1=bvexp[:].rearrange("p h u -> p (h u)"))
    y8 = sb(apool, "y8", (64, H, R), MMDT)
    nc.vector.tensor_mul(out=y8[:].rearrange("p h u -> p (h u)"),
                         in0=msgn[:],
                         in1=seT8[:].rearrange("p h u -> p (h u)"))
    xps = sb(ps_z, "ps_z", (R, ST), bufs=1)
    for h in range(H):
        nc.tensor.matmul(xps[:], y8[:, h, :], WeT8[:, h, :],
                         start=(h == 0), stop=(KT1 and h == H - 1))
    if not KT1:
        nc.tensor.matmul(xps[:], onesA[:1, :], brow[:, 1, :],
                         start=False, stop=True)
    x_att = xps

    cut(5)

    # ---- FFN ----
    bnf = sb(apool, "bnf", (R, 6))
    mvf = sb(apool, "mvf", (R, 2))
    nc.vector.bn_stats(out=bnf[:], in_=x_att[:])
    nc.vector.bn_aggr(out=mvf[:], in_=bnf[:])
    rstd_f = sb(apool, "rstd_f", (R, 1))
    nc.scalar.activation(out=rstd_f[:], in_=mvf[:, 1:2],
                         func=mybir.ActivationFunctionType.Sqrt, bias=epst[:R])
    nc.vector.reciprocal(out=rstd_f[:], in_=rstd_f[:])
    zf = sb(apool, "zf", (R, ST))
    nc.vector.tensor_scalar(out=zf[:], in0=x_att[:], scalar1=mvf[:, 0:1],
                            scalar2=rstd_f[:], op0=AXIS.subtract, op1=AXIS.mult)
    x1 = sb(apool, "x1", (R, ST), MMDT)
    nc.vector.tensor_mul(out=x1[:], in0=zf[:], in1=sf1g[:])
    nc.vector.tensor_add(out=x1[:], in0=x1[:], in1=bf1[:])
    x1T = sb(apool, "x1T", (128, 4, R), MMDT)
    for t in range(4):
        p = sb(ps_g, "gt", (128, 128), MMDT, bufs=1)
        transpose(p[:, :R], x1[:, t * 128:(t + 1) * 128])
        nc.any.tensor_copy(out=x1T[:, t, :], in_=p[:, :R])
    h1ps = sb(ps_g, "g", (R, HID))
    for t in range(4):
        nc.tensor.matmul(h1ps[:], x1T[:, t, :], W1T[:, t, :],
                         start=(t == 0), stop=(KT1 and t == 3))
    if not KT1:
        nc.tensor.matmul(h1ps[:], onesA[:1, :], brow[:, 2, :],
                         start=False, stop=True)
    h1g = sb(apool, "h1g", (R, HID), MMDT)
    _gelu = (mybir.ActivationFunctionType.Identity
             if _osK.environ.get("SIM_GELU_ID") == "1"
             else mybir.ActivationFunctionType.Gelu)
    nc.scalar.activation(out=h1g[:], in_=h1ps[:], func=_gelu)
    h1s = sb(apool, "h1s", (R, HID), MMDT)
    nc.vector.tensor_mul(out=h1s[:], in0=h1g[:], in1=scales["f2"][:])
    h1sT = sb(apool, "h1sT", (128, 4, R), MMDT)
    for t in range(4):
        p = sb(ps_g, "gt", (128, 128), MMDT, bufs=1)
        transpose(p[:, :R], h1s[:, t * 128:(t + 1) * 128])
        nc.any.tensor_copy(out=h1sT[:, t, :], in_=p[:, :R])
    h2ps = sb(ps_g, "g", (R, ST))
    for t in range(4):
        nc.tensor.matmul(h2ps[:], h1sT[:, t, :], W2T[:, t, :],
                         start=(t == 0), stop=(KT1 and t == 3))
    if not KT1:
        nc.tensor.matmul(h2ps[:], onesA[:1, :], brow[:, 3, :],
                         start=False, stop=True)
    o_sb = sb(apool, "o_sb", (R, ST))
    nc.vector.tensor_mul(out=o_sb[:], in0=h2ps[:], in1=bc["ls_ffn"])
    nc.vector.tensor_add(out=o_sb[:], in0=o_sb[:], in1=x_att[:])
    nc.sync.dma_start(out=out.ap(), in_=o_sb[:])


_NC_CACHE = None


def _get_nc():
    global _NC_CACHE
    if _NC_CACHE is None:
        nc = bacc.Bacc("TRN2", target_bir_lowering=False, debug=False,
                       num_devices=N_CORES)
        _NC_CACHE = _build(nc)
    return _NC_CACHE


def make_in_maps(inputs):
    f = lambda x: np.ascontiguousarray(np.asarray(x, np.float32), dtype=NPDT)
    i = {k: np.asarray(v, np.float32) for k, v in inputs.items()}
    pm = lambda x: f(np.transpose(x, (1, 0, 2)))      # (k,128,D)->(128,k,D)
    ls_a = i["ls_attn"]
    WeP = i["We"] * ls_a[:, None]                      # fold ls_attn
    pack64 = np.stack([i["ln_r_g"], i["ln_r_b"], i["ln_f_g"], i["ln_f_b"],
                       i["ls_ffn"]])                   # (5, 512)
    pack128 = np.stack([i["ln_s_g"], i["ln_s_b"]])     # (2, 256)
    brow = np.stack([i["bq"], i["be"] * ls_a, i["b1"], i["b2"]])  # (4, 512)
    parts = {
        "codesT": pm(i["receiver_codes"].reshape(R, CODE).T.reshape(2, 128, R)),
        "CqT": pm(i["Cq"].T.reshape(2, 128, ST)),
        "CkT": pm(i["Ck"].T.reshape(2, 128, IN)),
        "CvT": pm(i["Cv"].T.reshape(2, 128, IN)),
        "CeT": pm(i["Ce"].T.reshape(2, 128, ST)),
        "C1T": pm(i["C1"].T.reshape(2, 128, ST)),
        "C2T": pm(i["C2"].T.reshape(2, 128, HID)),
        "WqT": pm(i["Wq"].T.reshape(4, 128, INNER)),
        "Wk": pm(i["Wk"].reshape(4, 128, IN)),
        "WvT": pm(i["Wv"].T.reshape(2, 128, INNER)),
        "W1T": pm(i["W1"].T.reshape(4, 128, HID)),
        "W2T": pm(i["W2"].T.reshape(4, 128, ST)),
    }
    mega = np.concatenate([parts[nm].reshape(128, -1) for nm, _, _ in MEGA],
                          axis=1)
    assert mega.shape == (128, MEGA_F)
    common = {
        "recv": f(i["receiver_states"].reshape(R, ST)),
        "mega": f(mega),
        "WeT8": pm(WeP.T.reshape(8, 64, ST)),
        "pack64": f(np.broadcast_to(pack64[None], (R, 5, ST))),
        "pack128": f(np.broadcast_to(pack128[None], (128, 2, IN))),
        "brow": f(brow[None]),
        "bvexp": f(np.broadcast_to(i["bv"].reshape(8, 64).T[:, :, None],
                                   (64, 8, 64))),
        "sel4": f(np.eye(4)[:, :, None] * np.ones((1, 1, 64))),
        "sel4": f(np.eye(4)[:, :, None] * np.ones((1, 1, 64))),
    }
    in_maps = []
    for c in range(N_CORES):
        m = dict(common)
        shard = i["sender_states"][:, c * VC:(c + 1) * VC, :]     # (B, VC, IN)
        m["send"] = pm(shard.reshape(S, IN).reshape(4, 128, IN))
        in_maps.append(m)
    return in_maps


def kernel(**inputs) -> np.ndarray:
    nc = _get_nc()
    in_maps = make_in_maps(inputs)
    res = bass_utils.run_bass_kernel_spmd(nc, in_maps,
                                          core_ids=list(range(N_CORES)))
    return res.results[0]["out"].reshape(B, U, ST).astype(np.float32)
